# revision 1
# baseline (speedup 1.0000x reference)
"""Trainium2 Bass kernel for the DependencyTreeLSTM node-reduction step.

Contract: kernel(**inputs) takes the FULL (unsharded) numpy inputs exactly as
produced by setup_inputs() and returns the FULL [B, 2*SIZE] float32 output.

Strategy (8 NeuronCores, data-parallel over the node axis, no collectives):
  - Each core owns B/8 = 2048 nodes (= 32768 children rows).
  - Only the h-half of `children` is needed in bulk (the c-half matters only
    for the first 16 rows, see below). It is staged bf16 in a tiled row
    order so every DMA partition line is contiguous; sums accumulate in
    fp32 (PSUM / DVE pipeline). Measured end-to-end error vs the fp32
    reference: 5.1e-3 scale-relative max (1.9e-3 L2), gate is 2e-2.
  - Per-node sum over 16 children, split across engines to balance load:
    even node-tiles via TensorE matmuls with a 0/1 selection strip as the
    stationary operand, odd node-tiles via VectorE bf16 tree-adds (their
    rows staged feature-major so the adds run in the packed 2x mode).
    Sums are transposed feature-major with PE identity transposes.
  - iou = [sum_h/16, tracking_h, 1] @ [W_iou/16; W_iou_track; b_iou] on PE,
    sigmoid/tanh on ScalarE, elementwise on VectorE, node-major DMA out.
  - The reference computes fc_b = cumsum(fc)[lens-1]; with lens == 16
    everywhere this is one shared prefix over the first 16 rows of fc.
    Each core recomputes that tiny [1, 256] vector on device (in
    float32r) and broadcasts it with a K=1 ones outer-product matmul.

If the inputs do not match the structural assumptions (uniform 16-child
segments), we fall back to a plain numpy implementation of the reference
(never taken for the benchmark inputs).
"""

import sys

if "/opt/trn_rl_repo" not in sys.path:
    sys.path.insert(0, "/opt/trn_rl_repo")

import numpy as np

B = 16384
CH = 16
T = B * CH
SIZE = 256
TR = 256
NCORES = 8
B_LOC = B // NCORES          # 2048 nodes per core
T_LOC = B_LOC * CH           # 32768 children rows per core
NT = B_LOC // 128            # 16 node-tiles of 128 nodes per core
CH_PLAN = [1, 1, 2, 2, 3, 3, 3, 1]  # children DMA chunk sizes
CH_QUEUE = ["sync", "gpsimd", "sync", "gpsimd", "sync", "gpsimd", "sync", "gpsimd"]
DVE_TILES = frozenset(range(1, 16, 2))  # odd tiles reduced on VectorE
OUT_PLAN = [4, 4, 4, 2, 1, 1]  # output DMA group sizes (node-tiles)

_cache = {}
_DVE_TILES_HOST = frozenset(range(1, 16, 2))


def _sigmoid(x):
    return 1.0 / (1.0 + np.exp(-x))


def _reference_np(children, tracking, W_iou, b_iou, W_f, b_f, W_iou_track,
                  W_f_track, segment_ids, lens):
    size = W_f.shape[0]
    nb = tracking.shape[0]
    tr_h = tracking[:, : tracking.shape[1] // 2]
    sums = np.zeros((nb, children.shape[1]), np.float32)
    np.add.at(sums, segment_ids, children)
    mean_h = (sums / lens[:, None].astype(np.float32))[:, :size]
    iou = mean_h @ W_iou + b_iou + tr_h @ W_iou_track
    i, o, u = np.split(iou, 3, axis=1)
    i, o, u = _sigmoid(i), _sigmoid(o), np.tanh(u)
    f = children[:, :size] @ W_f + b_f + (tr_h @ W_f_track)[segment_ids]
    fc = _sigmoid(f) * children[:, size:]
    cs = np.cumsum(fc, axis=0, dtype=np.float32)
    fc_b = cs[lens - 1]
    c = i * u + fc_b
    h = o * c
    return np.concatenate([h, c], axis=1).astype(np.float32)


def _build_nc():
    import concourse.tile as tile
    from concourse import bacc, mybir
    from concourse.masks import make_identity

    f32 = mybir.dt.float32
    f32r = mybir.dt.float32r
    bf16 = mybir.dt.bfloat16
    SIG = mybir.ActivationFunctionType.Sigmoid
    TANH = mybir.ActivationFunctionType.Tanh

    nc = bacc.Bacc("TRN2", target_bir_lowering=False, debug=False,
                   num_devices=NCORES)

    ch_h = nc.declare_dram_parameter("ch_h", [T_LOC, SIZE], bf16, isOutput=False)
    trk = nc.declare_dram_parameter("trk", [B_LOC, SIZE], bf16, isOutput=False)
    sel = nc.declare_dram_parameter("sel", [128, 248], bf16, isOutput=False)
    wbig = nc.declare_dram_parameter("wbig", [128, 4, 3 * SIZE], bf16, isOutput=False)
    brow = nc.declare_dram_parameter("brow", [1, 3 * SIZE], bf16, isOutput=False)
    onesb = nc.declare_dram_parameter("onesb", [1, 128], bf16, isOutput=False)
    xt5 = nc.declare_dram_parameter("xt5", [128, 5, CH], bf16, isOutput=False)
    wc5 = nc.declare_dram_parameter("wc5", [128, 5, SIZE], bf16, isOutput=False)
    chc16 = nc.declare_dram_parameter("chc16", [CH, SIZE], f32, isOutput=False)
    ones_in = nc.declare_dram_parameter("ones_in", [CH, 128], f32, isOutput=False)
    y = nc.declare_dram_parameter("y", [B_LOC, 2 * SIZE], bf16, isOutput=True)
    dbg = _cache.get("debug")
    if dbg:
        d_act = nc.declare_dram_parameter("d_act", [128, 3 * SIZE], f32,
                                          isOutput=True)
        d_bc = nc.declare_dram_parameter("d_bc", [128, SIZE], f32, isOutput=True)
        d_zt = nc.declare_dram_parameter("d_zt", [128, 2, 128], f32,
                                         isOutput=True)

    # children staged host-side in (t, p, j) row order so each partition's
    # DMA line is contiguous; chunked loads, big first, small last
    chv = ch_h[:].rearrange("(t p j) d -> p t j d", p=128, j=CH)
    trkv = trk[:].rearrange("(t p) d -> p t d", p=128)
    assert sum(CH_PLAN) == NT
    yv = y[:].rearrange("(t p) d -> p t d", p=128)

    with tile.TileContext(nc) as tc:
        with (
            tc.tile_pool(name="consts", bufs=1) as consts,
            tc.tile_pool(name="chpool", bufs=3) as chpool,
            tc.tile_pool(name="sumpool", bufs=3) as sumpool,
            tc.tile_pool(name="ztpool", bufs=3) as ztpool,
            tc.tile_pool(name="actpool", bufs=3) as actpool,
            tc.tile_pool(name="outpool", bufs=2) as outpool,
            tc.tile_pool(name="psum_s", bufs=2, space="PSUM") as psum_s,
            tc.tile_pool(name="psum_t", bufs=2, space="PSUM") as psum_t,
            tc.tile_pool(name="psum_i", bufs=2, space="PSUM") as psum_i,
        ):
            # ---- constants (prefix-chain deps first, so PE starts early) --
            xt_sb = consts.tile([128, 5, CH], bf16)
            nc.scalar.dma_start(out=xt_sb, in_=xt5[:])
            wc_sb = consts.tile([128, 5, SIZE], bf16)
            nc.scalar.dma_start(out=wc_sb, in_=wc5[:])
            chc_sb = consts.tile([CH, SIZE], f32)
            nc.scalar.dma_start(out=chc_sb, in_=chc16[:])
            ones_sb = consts.tile([CH, 128], f32r)
            nc.scalar.dma_start(out=ones_sb, in_=ones_in[:].bitcast(f32r))
            ones1 = ones_sb[0:1, :]
            ones16 = ones_sb[:, 0:1]
            sel_sb = consts.tile([128, 248], bf16)
            nc.gpsimd.dma_start(out=sel_sb, in_=sel[:])
            # tracking, node-major; transposed per-tile on the PE
            trk_all = consts.tile([128, NT, SIZE], bf16)
            nc.gpsimd.dma_start(out=trk_all, in_=trkv)
            id_sb = consts.tile([128, 128], bf16)
            make_identity(nc, id_sb)
            w_sb = consts.tile([128, 4, 3 * SIZE], bf16)
            nc.scalar.dma_start(out=w_sb, in_=wbig[:])
            brow_sb = consts.tile([1, 3 * SIZE], bf16)
            nc.scalar.dma_start(out=brow_sb, in_=brow[:])
            ones1b = consts.tile([1, 128], bf16)
            nc.scalar.dma_start(out=ones1b, in_=onesb[:])

            # ---- fc prefix: fc_b = sum_{t<16} sigmoid(X @ Wcat)[t] * ch_c[t]

            psum_f = psum_t.tile([CH, SIZE], f32, tag="tr")
            for b in range(4):
                nc.tensor.matmul(psum_f, lhsT=xt_sb[:, b, :],
                                 rhs=wc_sb[:, b, :],
                                 start=(b == 0), stop=False)
            nc.tensor.matmul(psum_f, lhsT=xt_sb[0:1, 4, :],
                             rhs=wc_sb[0:1, 4, :],
                             start=False, stop=True)
            sig_sb = consts.tile([CH, SIZE], f32)
            nc.scalar.activation(out=sig_sb, in_=psum_f, func=SIG)
            fc_sb = consts.tile([CH, SIZE], f32r)
            nc.vector.tensor_mul(fc_sb, sig_sb, chc_sb)
            psum_pref = psum_t.tile([1, SIZE], f32, tag="tr")
            nc.tensor.matmul(psum_pref, lhsT=ones16,
                             rhs=fc_sb[:], start=True, stop=True)
            pref_sb = consts.tile([1, SIZE], f32r)
            nc.vector.tensor_copy(pref_sb, psum_pref)
            psum_bc = psum_t.tile([128, SIZE], f32, tag="tr")
            nc.tensor.matmul(psum_bc, lhsT=ones1,
                             rhs=pref_sb[:], start=True, stop=True)
            bc_sb = consts.tile([128, SIZE], f32)
            nc.vector.tensor_copy(bc_sb, psum_bc)
            if dbg:
                nc.scalar.dma_start(out=d_bc[:], in_=bc_sb)

            # ---- main loop over node-tiles ----
            chunk_of = []
            for ci, n in enumerate(CH_PLAN):
                chunk_of += [(ci, hi, n) for hi in range(n)]
            chunk_starts = [sum(CH_PLAN[:ci]) for ci in range(len(CH_PLAN))]
            ogrp_of = []
            for ui, n in enumerate(OUT_PLAN):
                ogrp_of += [(ui, hi, n) for hi in range(n)]
            ogrp_starts = [sum(OUT_PLAN[:ui]) for ui in range(len(OUT_PLAN))]
            assert sum(OUT_PLAN) == NT
            ch_sbs = {}
            out_grps = {}
            for t in range(NT):
                ci, hh, n = chunk_of[t]
                if hh == 0:
                    t0 = chunk_starts[ci]
                    ch_sbn = chpool.tile([128, max(CH_PLAN), CH, SIZE], bf16,
                                         name=f"ch{ci}", tag="ch")
                    dma_eng = {"sync": nc.sync, "scalar": nc.scalar,
                               "gpsimd": nc.gpsimd}[CH_QUEUE[ci]]
                    dma_eng.dma_start(out=ch_sbn[:, :n],
                                      in_=chv[:, t0:t0 + n])
                    ch_sbs[ci] = ch_sbn
                ch_sb = ch_sbs[ci][:, hh]

                # segment sum: sums[node, d] = sum of the node's 16 children.
                # Even tiles go through the PE (0/1 selection matmuls, rows on
                # partitions); odd tiles are staged feature-major per node and
                # reduced on the VectorE (bf16 2x mode), splitting the load.
                sums_sb = sumpool.tile([128, SIZE], bf16, name=f"sm{t}", tag="sm")
                if t in DVE_TILES:
                    # bf16 tree reduction over the child axis (innermost, so
                    # the adds run in the DVE 2x packed mode)
                    chview = ch_sb.rearrange("p a b -> p (a b)").rearrange(
                        "p (d j) -> p d j", j=CH)
                    tr8 = sumpool.tile([128, SIZE, 8], bf16, name=f"tr8_{t}",
                                       tag="tr8")
                    nc.vector.tensor_add(tr8, chview[:, :, 0:8],
                                         chview[:, :, 8:16])
                    tr4 = sumpool.tile([128, SIZE, 4], bf16, name=f"tr4_{t}",
                                       tag="tr4")
                    nc.vector.tensor_add(tr4, tr8[:, :, 0:4], tr8[:, :, 4:8])
                    tr2 = sumpool.tile([128, SIZE, 2], bf16, name=f"tr2_{t}",
                                       tag="tr2")
                    nc.vector.tensor_add(tr2, tr4[:, :, 0:2], tr4[:, :, 2:4])
                    nc.vector.tensor_add(sums_sb, tr2[:, :, 0:1], tr2[:, :, 1:2])
                else:
                    psum_sum = psum_s.tile([128, SIZE], f32, name=f"ps{t}",
                                           tag="ps")
                    for j in range(CH):
                        nc.tensor.matmul(psum_sum,
                                         lhsT=sel_sb[:, 120 - 8 * j:248 - 8 * j],
                                         rhs=ch_sb[:, j, :],
                                         start=(j == 0), stop=(j == CH - 1))
                    nc.vector.tensor_copy(sums_sb, psum_sum)

                # transpose sums and tracking to feature-major K blocks
                zt_sb = ztpool.tile([128, 4, 128], bf16, name=f"zt{t}", tag="zt")
                psum_T = psum_t.tile([128, 4, 128], bf16, name=f"pm{t}", tag="tr")
                nc.tensor.transpose(psum_T[:, 0, :], sums_sb[:, 0:128], id_sb)
                nc.tensor.transpose(psum_T[:, 1, :], sums_sb[:, 128:256], id_sb)
                nc.tensor.transpose(psum_T[:, 2, :], trk_all[:, t, 0:128], id_sb)
                nc.tensor.transpose(psum_T[:, 3, :], trk_all[:, t, 128:256], id_sb)
                nc.vector.tensor_copy(zt_sb, psum_T)

                # iou[node, 0:768] = Z @ [W_iou/16; W_iou_track] + b_iou
                psum_iou = psum_i.tile([128, 3 * SIZE], f32, name=f"pi{t}", tag="pi")
                for c0, cn in ((0, 512), (512, 256)):
                    cs = slice(c0, c0 + cn)
                    nc.tensor.matmul(psum_iou[:, cs], lhsT=ones1b,
                                     rhs=brow_sb[:, cs], start=True, stop=False)
                    # tracking K-blocks first: they don't depend on this
                    # tile's segment sum, so PE can start iou early
                    for b in (2, 3, 0, 1):
                        nc.tensor.matmul(psum_iou[:, cs], lhsT=zt_sb[:, b, :],
                                         rhs=w_sb[:, b, cs],
                                         start=False, stop=(b == 1))

                act_sb = actpool.tile([128, 3 * SIZE], f32, name=f"ac{t}", tag="ac")
                nc.scalar.activation(out=act_sb[:, 0:512],
                                     in_=psum_iou[:, 0:512], func=SIG)
                nc.scalar.activation(out=act_sb[:, 512:768],
                                     in_=psum_iou[:, 512:768], func=TANH)

                if dbg and t == 5:
                    act_f = actpool.tile([128, 3 * SIZE], f32, name="dbg_act",
                                         tag="dbg")
                    nc.vector.tensor_copy(act_f, act_sb)
                    nc.scalar.dma_start(out=d_act[:], in_=act_f)
                    zt_f = ztpool.tile([128, 2, 128], f32, name="dbg_zt",
                                       tag="dbgz")
                    nc.vector.tensor_copy(zt_f, zt_sb)
                    nc.scalar.dma_start(out=d_zt[:], in_=zt_f)
                u, gh, gn = ogrp_of[t]
                if gh == 0:
                    out_grps[u] = outpool.tile([128, max(OUT_PLAN), 2 * SIZE],
                                               bf16, name=f"ot{u}", tag="ot")
                out_sb = out_grps[u][:, gh]
                # c = i*u + fc_b ; h = o*c
                nc.vector.tensor_mul(out_sb[:, 256:512], act_sb[:, 0:256],
                                     act_sb[:, 512:768])
                nc.vector.tensor_add(out_sb[:, 256:512], out_sb[:, 256:512],
                                     bc_sb)
                nc.vector.tensor_mul(out_sb[:, 0:256], act_sb[:, 256:512],
                                     out_sb[:, 256:512])
                if gh == gn - 1:
                    g0 = ogrp_starts[u]
                    nc.gpsimd.dma_start(out=yv[:, g0:g0 + gn],
                                        in_=out_grps[u][:, :gn])

    nc.finalize()
    return nc


def _get_nc():
    if "nc" not in _cache:
        _cache["nc"] = _build_nc()
    return _cache["nc"]


def kernel(**inputs):
    import ml_dtypes

    bf16 = ml_dtypes.bfloat16

    children = np.ascontiguousarray(np.asarray(inputs["children"], np.float32))
    tracking = np.ascontiguousarray(np.asarray(inputs["tracking"], np.float32))
    W_iou = np.asarray(inputs["W_iou"], np.float32)
    b_iou = np.asarray(inputs["b_iou"], np.float32)
    W_f = np.asarray(inputs["W_f"], np.float32)
    b_f = np.asarray(inputs["b_f"], np.float32)
    W_iou_track = np.asarray(inputs["W_iou_track"], np.float32)
    W_f_track = np.asarray(inputs["W_f_track"], np.float32)
    segment_ids = np.asarray(inputs["segment_ids"], np.int32)
    lens = np.asarray(inputs["lens"], np.int32)

    structured = (
        children.shape == (T, 2 * SIZE)
        and tracking.shape == (B, 2 * TR)
        and W_iou.shape == (SIZE, 3 * SIZE)
        and W_f.shape == (SIZE, SIZE)
        and W_iou_track.shape == (TR, 3 * SIZE)
        and W_f_track.shape == (TR, SIZE)
        and lens.shape == (B,)
        and segment_ids.shape == (T,)
        and bool((lens == CH).all())
        and bool((segment_ids == np.repeat(np.arange(B, dtype=np.int32), CH)).all())
    )
    if not structured:
        return _reference_np(children, tracking, W_iou, b_iou, W_f, b_f,
                             W_iou_track, W_f_track, segment_ids, lens)

    from concourse.bass_utils import run_bass_kernel_spmd

    nc = _get_nc()
    in_maps = _stage_in_maps(children, tracking, W_iou, b_iou, W_f, b_f,
                             W_iou_track, W_f_track, segment_ids)

    res = run_bass_kernel_spmd(nc, in_maps, core_ids=list(range(NCORES)))
    _cache["last_exec_time_ns"] = res.exec_time_ns
    out = np.concatenate([np.asarray(r["y"]).astype(np.float32)
                          for r in res.results], axis=0)
    return out


def _stage_in_maps(children, tracking, W_iou, b_iou, W_f, b_f,
                   W_iou_track, W_f_track, segment_ids):
    import ml_dtypes

    bf16 = ml_dtypes.bfloat16
    tr_h = tracking[:, :TR]

    # selection strip: strip[r, x] = 1 iff x == r//16 + 120, so that the
    # slice strip[:, 120-8k : 248-8k] is the k-th 0/1 selection matrix
    r = np.arange(128)
    sel = np.zeros((128, 248), np.float32)
    sel[r, r // 16 + 120] = 1.0

    # fused iou weight [mean(/16) ; tracking] blocks, K-major tiles of 128
    wcat = np.concatenate([W_iou / np.float32(16.0), W_iou_track], axis=0)
    wbig = np.ascontiguousarray(
        wcat.reshape(4, 128, 3 * SIZE).transpose(1, 0, 2).astype(bf16))
    brow = np.ascontiguousarray(b_iou.reshape(1, 3 * SIZE).astype(bf16))

    # prefix-f inputs: X = [ch_h[0:16], trk_h[seg[0:16]], 1],
    # W = [W_f; W_f_track; b_f] (kept f32/f32r)
    X = np.concatenate([
        children[:CH, :SIZE],
        tr_h[segment_ids[:CH]],
        np.ones((CH, 1), np.float32),
    ], axis=1)                                       # [16, 513]
    XT = np.zeros((5 * 128, CH), np.float32)
    XT[: 2 * SIZE + 1] = X.T
    xt5 = np.ascontiguousarray(
        XT.reshape(5, 128, CH).transpose(1, 0, 2).astype(bf16))
    WC = np.zeros((5 * 128, SIZE), np.float32)
    WC[:SIZE] = W_f
    WC[SIZE: 2 * SIZE] = W_f_track
    WC[2 * SIZE] = b_f
    wc5 = np.ascontiguousarray(
        WC.reshape(5, 128, SIZE).transpose(1, 0, 2).astype(bf16))
    chc16 = np.ascontiguousarray(children[:CH, SIZE:])

    shared = {"sel": sel.astype(bf16), "wbig": wbig, "brow": brow,
              "onesb": np.ones((1, 128), bf16),
              "xt5": xt5, "wc5": wc5, "chc16": chc16,
              "ones_in": np.ones((CH, 128), np.float32)}
    in_maps = []
    for c in range(NCORES):
        shard = children[c * T_LOC:(c + 1) * T_LOC, :SIZE].astype(bf16)
        shard = shard.reshape(NT, 128, CH, SIZE)     # [t, node, child, feat]
        staged = np.empty((NT, 128, CH * SIZE), shard.dtype)
        for t in range(NT):
            if t in _DVE_TILES_HOST:
                # node on partitions, feature-major within node: [n][d][j]
                staged[t] = shard[t].transpose(0, 2, 1).reshape(128, CH * SIZE)
            else:
                # child-row r = j*128+p on partitions p, blocks j along free:
                # [p][j][d] from original (node, child)-major rows
                staged[t] = (shard[t].reshape(CH, 128, SIZE)
                             .transpose(1, 0, 2).reshape(128, CH * SIZE))
        in_maps.append({
            "ch_h": np.ascontiguousarray(staged.reshape(T_LOC, SIZE)),
            "trk": np.ascontiguousarray(
                tr_h[c * B_LOC:(c + 1) * B_LOC].astype(bf16)),
            **shared,
        })

    return in_maps



# revision 68
# speedup vs baseline: 1.9065x; 1.9065x over previous
"""Trainium2 Bass kernel for the DependencyTreeLSTM node-reduction step.

Contract: kernel(**inputs) takes the FULL (unsharded) numpy inputs exactly as
produced by setup_inputs() and returns the FULL [B, 2*SIZE] float32 output.

Strategy (8 NeuronCores, data-parallel over the node axis, no collectives):
  - Each core owns B/8 = 2048 nodes (= 32768 children rows).
  - Only the h-half of `children` is needed in bulk (the c-half matters only
    for the first 16 rows, see below). It is staged **fp8 e4m3**: the
    children stream is ~2/3 of all HBM traffic and fp8 halves it; the
    per-node means average 16 children so the quantization noise largely
    cancels (measured ~9e-3 scale-relative max error, gate is 2e-2).
  - The children stream is split over the three DMA queues (SP-HWDGE,
    ACT-HWDGE, Pool-SWDGE), which transfer concurrently; each queue is
    internally FIFO so chunk order is hand-scheduled around need times.
  - Segment mean per 128-node tile: 8 DoubleRow fp8 matmuls (2x128 child
    rows each) with a 0/(1/16) selection strip stationary -> PSUM holds the
    mean directly (the /16 lives in the strip values).
  - mean is transposed feature-major with PE identity transposes, quantized
    to fp8, and iou = [mean, tracking_h, 1] @ [W_iou; W_iou_track; b_iou]
    runs on PE: the mean block as fp8 DoubleRow, the tracking block in bf16
    (tracking in fp8 fails the accuracy gate - measured 2.9e-2), the bias
    as a K=1 DoubleRow outer product with a (1/16, 16*b) scale split.
  - sigmoid/tanh on ScalarE, elementwise on VectorE, node-major DMA out.
  - The reference computes fc_b = cumsum(fc)[lens-1]; with lens == 16
    everywhere this is one shared prefix over the first 16 rows of fc.
    Each core recomputes that tiny [1, 256] vector on device and
    broadcasts it with a K=1 ones outer-product matmul.

If the inputs do not match the structural assumptions (uniform 16-child
segments), we fall back to a plain numpy implementation of the reference
(never taken for the benchmark inputs).
"""

import sys

if "/opt/trn_rl_repo" not in sys.path:
    sys.path.insert(0, "/opt/trn_rl_repo")

import numpy as np

B = 16384
CH = 16
T = B * CH
SIZE = 256
TR = 256
NCORES = 8
B_LOC = B // NCORES          # 2048 nodes per core
T_LOC = B_LOC * CH           # 32768 children rows per core
NT = B_LOC // 128            # 16 node-tiles of 128 nodes per core
NJ = 8                       # DoubleRow matmuls per node-tile (256 rows each)
# children chunks: (queue, tiles). The sync HWDGE queue takes the earliest
# tiles (fast handoff); the SWDGE queue takes the bulk that is needed later.
CH_CHUNKS = [("sync", [0]), ("gpsimd", [1]), ("sync", [2, 3]),
             ("sync", [4, 5]), ("gpsimd", [6, 7]), ("gpsimd", [8, 9]),
             ("sync", [10, 11]), ("gpsimd", [12, 13]), ("sync", [14]),
             ("gpsimd", [15])]
# tile index -> chunk ids whose DMA fires at the top of that tile
CH_ISSUE = {0: [0, 1], 1: [2], 2: [3, 4], 5: [5], 7: [6], 8: [7], 9: [9],
            11: [8]}
TRK_SPLIT = 2
import os as _os, json as _json
if _os.environ.get("SCHED_JSON"):
    _sj = _json.loads(_os.environ["SCHED_JSON"])
    CH_CHUNKS = [(q, ts) for q, ts in _sj["chunks"]]
    CH_ISSUE = {int(k): v for k, v in _sj["issue"].items()}
    TRK_SPLIT = _sj.get("trk", TRK_SPLIT)                # tracking tiles on sync before the rest on pool
OUT_PLAN = [4, 4, 4, 3, 1]   # output DMA groups; small tail groups
Y_ISSUE = {6: 0, 10: 1, 12: 2, 14: 3, 15: 4}  # tile at which y-group fires

_cache = {}


def _sigmoid(x):
    return 1.0 / (1.0 + np.exp(-x))


def _reference_np(children, tracking, W_iou, b_iou, W_f, b_f, W_iou_track,
                  W_f_track, segment_ids, lens):
    size = W_f.shape[0]
    nb = tracking.shape[0]
    tr_h = tracking[:, : tracking.shape[1] // 2]
    sums = np.zeros((nb, children.shape[1]), np.float32)
    np.add.at(sums, segment_ids, children)
    mean_h = (sums / lens[:, None].astype(np.float32))[:, :size]
    iou = mean_h @ W_iou + b_iou + tr_h @ W_iou_track
    i, o, u = np.split(iou, 3, axis=1)
    i, o, u = _sigmoid(i), _sigmoid(o), np.tanh(u)
    f = children[:, :size] @ W_f + b_f + (tr_h @ W_f_track)[segment_ids]
    fc = _sigmoid(f) * children[:, size:]
    cs = np.cumsum(fc, axis=0, dtype=np.float32)
    fc_b = cs[lens - 1]
    c = i * u + fc_b
    h = o * c
    return np.concatenate([h, c], axis=1).astype(np.float32)


def _build_nc():
    import concourse.tile as tile
    from concourse import bacc, mybir

    f32 = mybir.dt.float32
    f32r = mybir.dt.float32r
    bf16 = mybir.dt.bfloat16
    fp8 = mybir.dt.float8e4
    SIG = mybir.ActivationFunctionType.Sigmoid
    TANH = mybir.ActivationFunctionType.Tanh
    COPY = mybir.ActivationFunctionType.Copy
    DR = mybir.MatmulPerfMode.DoubleRow

    nc = bacc.Bacc("TRN2", target_bir_lowering=False, debug=False,
                   num_devices=NCORES)

    # children h-half, fp8, staged (t, p, j, i*d) so each partition's DMA
    # line is one contiguous 4 KB run per node-tile
    ch8 = nc.declare_dram_parameter("ch8", [T_LOC // 2, 2 * SIZE], fp8,
                                    isOutput=False)
    # tracking h transposed, three fp8 streams: hi = q(trk),
    # lo = q(16*(trk - hi)), q4 = q(trk/4); with the matching W streams the
    # three DoubleRow groups reproduce trk @ (16*W_track) to ~bf16 accuracy
    th8 = nc.declare_dram_parameter("th8", [NT * 128, 2, 128], fp8,
                                    isOutput=False)
    tl8 = nc.declare_dram_parameter("tl8", [NT * 128, 2, 128], fp8,
                                    isOutput=False)
    tq8 = nc.declare_dram_parameter("tq8", [NT * 128, 2, 128], fp8,
                                    isOutput=False)
    # selection strip: strip[p, i, x] = 1/16 iff x == 8i + p//16 + 112;
    # columns [240:368) hold the DR-bias lhs (partition 0: half0=1, half1=0)
    sel8 = nc.declare_dram_parameter("sel8", [128, 2, 368], fp8, isOutput=False)
    # W_iou as a DoubleRow K-group: wdr[p, i, n] = W_iou[i*128+p, n]
    wdr = nc.declare_dram_parameter("wdr", [128, 2, 3 * SIZE], fp8,
                                    isOutput=False)
    # W_iou_track fp8 K-groups: hi = q(16W), raw = q(W),
    # res = q(4*(16W - hi))
    wt1 = nc.declare_dram_parameter("wt1", [128, 2, 3 * SIZE], fp8,
                                    isOutput=False)
    wt2 = nc.declare_dram_parameter("wt2", [128, 2, 3 * SIZE], fp8,
                                    isOutput=False)
    wt3 = nc.declare_dram_parameter("wt3", [128, 2, 3 * SIZE], fp8,
                                    isOutput=False)
    # bias as DR rhs: row 0 = 16*b_iou, row 1 = 0 (lhs supplies the 1/16)
    bias2 = nc.declare_dram_parameter("bias2", [1, 2, 3 * SIZE], fp8,
                                      isOutput=False)
    # fc prefix inputs packed into one tensor: block b<4 holds the K-block
    # [X^T | Wcat] pair, block 4 row 0 holds the [ones | b_f] tail
    pfx = nc.declare_dram_parameter("pfx", [128, 5, CH + SIZE], bf16,
                                    isOutput=False)
    chc16 = nc.declare_dram_parameter("chc16", [CH, SIZE], f32, isOutput=False)
    y = nc.declare_dram_parameter("y", [B_LOC, 2 * SIZE], bf16, isOutput=True)

    chv = ch8[:].rearrange("(t p j) q -> p t j q", p=128, j=NJ)  # q = 2*256
    thv = th8[:].rearrange("(t p) i m -> p t i m", p=128)
    tlv = tl8[:].rearrange("(t p) i m -> p t i m", p=128)
    tqv = tq8[:].rearrange("(t p) i m -> p t i m", p=128)
    yv = y[:].rearrange("(t p) d -> p t d", p=128)
    assert sorted(sum((ts for _, ts in CH_CHUNKS), [])) == list(range(NT))
    assert sum(OUT_PLAN) == NT

    with tile.TileContext(nc) as tc:
        with (
            tc.tile_pool(name="consts", bufs=1) as consts,
            tc.tile_pool(name="chpool_a", bufs=3) as chpool_a,
            tc.tile_pool(name="chpool_b", bufs=3) as chpool_b,
            tc.tile_pool(name="ztpool", bufs=3) as ztpool,
            tc.tile_pool(name="actpool", bufs=3) as actpool,
            tc.tile_pool(name="outpool", bufs=2) as outpool,
            tc.tile_pool(name="psum_s", bufs=2, space="PSUM") as psum_s,
            tc.tile_pool(name="psum_i", bufs=3, space="PSUM") as psum_i,
        ):
            # ---- DMA schedule ----
            # sync  : ch{0}, ch{1}, trk[0:4], ch{4,5}, y0, ch{8,9}, y1,
            #         ch{14}, y2, ch{10,11}... (children issued in-loop)
            # gpsimd: sel, ch{2,3}, trk[4:16], ch{6,7}, ch{12,13}, ch{15}
            # scalar: pfx, chc16, wdr, wtk, bias
            th_sb = consts.tile([128, NT, 2, 128], fp8)
            tl_sb = consts.tile([128, NT, 2, 128], fp8)
            tq_sb = consts.tile([128, NT, 2, 128], fp8)
            sel_sb = consts.tile([128, 2, 368], fp8)
            nc.gpsimd.dma_start(out=sel_sb, in_=sel8[:])
            wdr_sb = consts.tile([128, 2, 3 * SIZE], fp8)
            nc.scalar.dma_start(out=wdr_sb, in_=wdr[:])
            wt1_sb = consts.tile([128, 2, 3 * SIZE], fp8)
            nc.scalar.dma_start(out=wt1_sb, in_=wt1[:])
            wt2_sb = consts.tile([128, 2, 3 * SIZE], fp8)
            nc.scalar.dma_start(out=wt2_sb, in_=wt2[:])
            wt3_sb = consts.tile([128, 2, 3 * SIZE], fp8)
            nc.scalar.dma_start(out=wt3_sb, in_=wt3[:])
            bias_sb = consts.tile([1, 2, 3 * SIZE], fp8)
            nc.scalar.dma_start(out=bias_sb, in_=bias2[:])
            pfx_sb = consts.tile([128, 5, CH + SIZE], bf16)
            chc_sb = consts.tile([CH, SIZE], f32)
            blhs_sb = sel_sb[0:1, :, 240:368]
            ones_f = consts.tile([CH, 128], f32)
            nc.vector.memset(ones_f, 1.0)
            ones_v = ones_f[:].bitcast(f32r)
            ones1 = ones_v[0:1, :]
            ones16 = ones_v[:, 0:1]

            bc_sb = consts.tile([128, SIZE], bf16)
            psum_f_box = []

            def emit_prefix():
                # fc prefix: fc_b = sum_{t<16} sigmoid(X @ Wcat)[t] * ch_c[t]
                # Emitted between tile 0's mean and its iou: the PE fills the
                # window where it would otherwise stall on the wtk/bias DMAs.
                psum_f = psum_i.tile([CH, SIZE], f32, tag="pi")
                for b in range(4):
                    nc.tensor.matmul(psum_f, lhsT=pfx_sb[:, b, 0:CH],
                                     rhs=pfx_sb[:, b, CH:],
                                     start=(b == 0), stop=False)
                nc.tensor.matmul(psum_f, lhsT=pfx_sb[0:1, 4, 0:CH],
                                 rhs=pfx_sb[0:1, 4, CH:],
                                 start=False, stop=True)
                psum_f_box.append(psum_f)

            def emit_prefix_reduce():
                psum_f = psum_f_box.pop()
                sig_sb = consts.tile([CH, SIZE], f32)
                nc.scalar.activation(out=sig_sb, in_=psum_f, func=SIG)
                fc_sb = consts.tile([CH, SIZE], f32r)
                nc.vector.tensor_mul(fc_sb, sig_sb, chc_sb)
                psum_pref = psum_i.tile([1, SIZE], f32, tag="pi")
                nc.tensor.matmul(psum_pref, lhsT=ones16,
                                 rhs=fc_sb[:], start=True, stop=True)
                pref_sb = consts.tile([1, SIZE], f32r)
                nc.vector.tensor_copy(pref_sb, psum_pref)
                psum_bc = psum_i.tile([128, SIZE], f32, tag="pi")
                nc.tensor.matmul(psum_bc, lhsT=ones1,
                                 rhs=pref_sb[:], start=True, stop=True)
                nc.vector.tensor_copy(bc_sb, psum_bc)

            # ---- main loop over node-tiles ----
            chunk_of = {}
            for ci, (q, ts) in enumerate(CH_CHUNKS):
                for hi, t in enumerate(ts):
                    chunk_of[t] = (ci, hi)
            nmax = max(len(ts) for _, ts in CH_CHUNKS)
            ogrp_of = []
            for ui, n in enumerate(OUT_PLAN):
                ogrp_of += [(ui, hi, n) for hi in range(n)]
            ogrp_starts = [sum(OUT_PLAN[:ui]) for ui in range(len(OUT_PLAN))]
            ch_sbs = {}
            out_grps = {}
            zt_sbs = {}

            def stage_sums(t):
                # fire children chunk DMAs scheduled at this tile index
                for ci in CH_ISSUE.get(t, ()):
                    q, ts = CH_CHUNKS[ci]
                    pool = chpool_a if q == "sync" else chpool_b
                    ch_sbn = pool.tile([128, nmax, NJ, 2 * SIZE],
                                       fp8, name=f"ch{ci}", tag=f"ch_{q}")
                    dma_eng = {"sync": nc.sync, "gpsimd": nc.gpsimd}[q]
                    if ci == 0:
                        # split so the very first half-tile lands sooner
                        dma_eng.dma_start(out=ch_sbn[:, 0, 0:NJ // 2],
                                          in_=chv[:, 0, 0:NJ // 2])
                        dma_eng.dma_start(out=ch_sbn[:, 0, NJ // 2:],
                                          in_=chv[:, 0, NJ // 2:])
                    else:
                        dma_eng.dma_start(out=ch_sbn[:, :len(ts)],
                                          in_=chv[:, ts[0]:ts[0] + len(ts)])
                    ch_sbs[ci] = ch_sbn
                # prefix consts + tracking ride the sync queue early
                if t == 0:
                    for sb, v in ((th_sb, thv), (tl_sb, tlv), (tq_sb, tqv)):
                        nc.sync.dma_start(out=sb[:, 0:TRK_SPLIT],
                                          in_=v[:, 0:TRK_SPLIT])
                    nc.sync.dma_start(out=pfx_sb, in_=pfx[:])
                    nc.sync.dma_start(out=chc_sb, in_=chc16[:])
                elif t == 1:
                    for sb, v in ((th_sb, thv), (tl_sb, tlv), (tq_sb, tqv)):
                        nc.gpsimd.dma_start(out=sb[:, TRK_SPLIT:NT],
                                            in_=v[:, TRK_SPLIT:NT])
                ci, hh = chunk_of[t]
                ch_sb = ch_sbs[ci][:, hh]          # [128, NJ, 512]

                # segment mean, produced feature-major directly: children
                # blocks stationary, selection strip (values 1/16) moving.
                # out[d, node] = sum over the block's 256 child rows; 16
                # DoubleRow fp8 matmuls accumulate all 2048 rows.
                psum_sum = psum_s.tile([128, 2, 128], f32, name=f"ps{t}",
                                       tag="ps")
                for j in range(NJ):
                    chj = ch_sb[:, j].rearrange("p (i d) -> p i d", i=2)
                    selj = sel_sb[:, :, 112 - 16 * j:240 - 16 * j]
                    for h in range(2):
                        nc.tensor.matmul(psum_sum[:, h],
                                         lhsT=chj[:, :, 128 * h:128 * h + 128],
                                         rhs=selj,
                                         start=(j == 0 and h == 0),
                                         stop=(j == NJ - 1 and h == 1),
                                         perf_mode=DR)
                zt_sb = ztpool.tile([128, 2, 128], fp8, name=f"zt{t}", tag="zt")
                nc.vector.tensor_copy(zt_sb, psum_sum)
                zt_sbs[t] = zt_sb
                if t == 1:
                    emit_prefix()

            def stage_rest(t):
                zt_sb = zt_sbs.pop(t)
                # iou[node, 0:768] = mean @ W_iou + trk @ W_iou_track + b_iou
                # PSUM groups are per 2KB zero-region (512 f32 cols): one
                # start and one stop per region; DR moving capped at 256 cols
                psum_iou = psum_i.tile([128, 3 * SIZE], f32, name=f"pi{t}",
                                       tag="pi")
                for ck in range(3):
                    ds = slice(256 * ck, 256 * ck + 256)
                    first = ck in (0, 2)
                    last = ck in (1, 2)
                    for gi, (lhs, rhs) in enumerate((
                            (th_sb[:, t], wt1_sb), (tl_sb[:, t], wt2_sb),
                            (tq_sb[:, t], wt3_sb), (zt_sb, wdr_sb),
                            (blhs_sb, bias_sb))):
                        nc.tensor.matmul(psum_iou[:, ds], lhsT=lhs,
                                         rhs=rhs[:, :, ds],
                                         start=(first and gi == 0),
                                         stop=(last and gi == 4),
                                         perf_mode=DR)

                if t == 0:
                    emit_prefix_reduce()
                # PSUM holds 16x iou (weights staged pre-scaled by 16 so the
                # fp8 W_iou values stay clear of the subnormal range)
                act_sb = actpool.tile([128, 3 * SIZE], bf16, name=f"ac{t}",
                                      tag="ac")
                nc.scalar.activation(out=act_sb[:, 0:512],
                                     in_=psum_iou[:, 0:512], func=SIG,
                                     scale=1.0 / 16.0)
                nc.scalar.activation(out=act_sb[:, 512:768],
                                     in_=psum_iou[:, 512:768], func=TANH,
                                     scale=1.0 / 16.0)

                u, gh, gn = ogrp_of[t]
                if gh == 0:
                    out_grps[u] = outpool.tile([128, max(OUT_PLAN), 2 * SIZE],
                                               bf16, name=f"ot{u}", tag="ot")
                out_sb = out_grps[u][:, gh]
                # c = i*u + fc_b ; h = o*c
                nc.vector.tensor_mul(out_sb[:, 256:512], act_sb[:, 0:256],
                                     act_sb[:, 512:768])
                nc.vector.tensor_add(out_sb[:, 256:512], out_sb[:, 256:512],
                                     bc_sb)
                nc.vector.tensor_mul(out_sb[:, 0:256], act_sb[:, 256:512],
                                     out_sb[:, 256:512])
                # y groups ride the sync queue, issued late so they never
                # block a children chunk the compute still needs
                if t in Y_ISSUE:
                    u2 = Y_ISSUE[t]
                    g0 = ogrp_starts[u2]
                    gn2 = OUT_PLAN[u2]
                    nc.sync.dma_start(out=yv[:, g0:g0 + gn2],
                                      in_=out_grps[u2][:, :gn2])

            # software pipeline: sums run one tile ahead of iou/act/ew so PE
            # fills the initial weight-DMA wait with the next tile's sums
            for t in range(NT + 1):
                if t < NT:
                    stage_sums(t)
                if t >= 1:
                    stage_rest(t - 1)

    nc.finalize()
    return nc


def _get_nc():
    if "nc" not in _cache:
        _cache["nc"] = _build_nc()
    return _cache["nc"]


def kernel(**inputs):
    children = np.ascontiguousarray(np.asarray(inputs["children"], np.float32))
    tracking = np.ascontiguousarray(np.asarray(inputs["tracking"], np.float32))
    W_iou = np.asarray(inputs["W_iou"], np.float32)
    b_iou = np.asarray(inputs["b_iou"], np.float32)
    W_f = np.asarray(inputs["W_f"], np.float32)
    b_f = np.asarray(inputs["b_f"], np.float32)
    W_iou_track = np.asarray(inputs["W_iou_track"], np.float32)
    W_f_track = np.asarray(inputs["W_f_track"], np.float32)
    segment_ids = np.asarray(inputs["segment_ids"], np.int32)
    lens = np.asarray(inputs["lens"], np.int32)

    structured = (
        children.shape == (T, 2 * SIZE)
        and tracking.shape == (B, 2 * TR)
        and W_iou.shape == (SIZE, 3 * SIZE)
        and W_f.shape == (SIZE, SIZE)
        and W_iou_track.shape == (TR, 3 * SIZE)
        and W_f_track.shape == (TR, SIZE)
        and lens.shape == (B,)
        and segment_ids.shape == (T,)
        and bool((lens == CH).all())
        and bool((segment_ids == np.repeat(np.arange(B, dtype=np.int32), CH)).all())
    )
    if not structured:
        return _reference_np(children, tracking, W_iou, b_iou, W_f, b_f,
                             W_iou_track, W_f_track, segment_ids, lens)

    from concourse.bass_utils import run_bass_kernel_spmd

    nc = _get_nc()
    in_maps = _stage_in_maps(children, tracking, W_iou, b_iou, W_f, b_f,
                             W_iou_track, W_f_track, segment_ids)

    res = run_bass_kernel_spmd(nc, in_maps, core_ids=list(range(NCORES)))
    _cache["last_exec_time_ns"] = res.exec_time_ns
    out = np.concatenate([np.asarray(r["y"]).astype(np.float32)
                          for r in res.results], axis=0)
    return out


def _stage_in_maps(children, tracking, W_iou, b_iou, W_f, b_f,
                   W_iou_track, W_f_track, segment_ids):
    import ml_dtypes

    bf16 = ml_dtypes.bfloat16
    fp8 = ml_dtypes.float8_e4m3
    tr_h = tracking[:, :TR]

    # selection strip: strip[p, i, x] = 1/16 iff x == 8i + p//16 + 112, so
    # the slice strip[:, :, 112-16j : 240-16j] selects node 16j+8i+p//16
    p = np.arange(128)
    sel = np.zeros((128, 2, 368), np.float32)
    for i in range(2):
        sel[p, i, 8 * i + p // 16 + 112] = 1.0 / 16.0
    sel[0, 0, 240:368] = 1.0

    # weights pre-scaled by 16 (activation applies 1/16): keeps the fp8
    # W_iou values in the e4m3 normal range
    def kblocks(w):
        return np.ascontiguousarray(
            w.reshape(2, 128, 3 * SIZE).transpose(1, 0, 2)).astype(fp8)

    wdr = kblocks(16.0 * W_iou)
    WS = 16.0 * W_iou_track
    WS_hi = WS.astype(fp8).astype(np.float32)
    wt1 = kblocks(WS)
    wt2 = kblocks(W_iou_track)
    wt3 = kblocks(4.0 * (WS - WS_hi))
    bias2 = np.zeros((1, 2, 3 * SIZE), np.float32)
    bias2[0, 0] = 16.0 * b_iou

    # prefix-f inputs: X = [ch_h[0:16], trk_h[seg[0:16]], 1],
    # W = [W_f; W_f_track; b_f], packed K-blocks [X^T | Wcat]
    X = np.concatenate([
        children[:CH, :SIZE],
        tr_h[segment_ids[:CH]],
        np.ones((CH, 1), np.float32),
    ], axis=1)                                       # [16, 513]
    XT = X.T                                         # [513, 16]
    WC = np.concatenate([W_f, W_f_track], axis=0)    # [512, 256]
    pfx = np.zeros((128, 5, CH + SIZE), np.float32)
    for b in range(4):
        pfx[:, b, :CH] = XT[b * 128:(b + 1) * 128]
        pfx[:, b, CH:] = WC[b * 128:(b + 1) * 128]
    pfx[0, 4, :CH] = 1.0
    pfx[0, 4, CH:] = b_f
    chc16 = np.ascontiguousarray(children[:CH, SIZE:])

    shared = {"sel8": sel.astype(fp8), "wdr": wdr, "wt1": wt1, "wt2": wt2,
              "wt3": wt3, "bias2": bias2.astype(fp8),
              "pfx": pfx.astype(bf16), "chc16": chc16}
    in_maps = []
    for c in range(NCORES):
        shard = children[c * T_LOC:(c + 1) * T_LOC, :SIZE].astype(fp8)
        # staged[t, p, j, i, d] = shard[t*2048 + j*256 + i*128 + p, d]
        staged = np.ascontiguousarray(
            shard.reshape(NT, NJ, 2, 128, SIZE).transpose(0, 3, 1, 2, 4))
        trk_loc = tr_h[c * B_LOC:(c + 1) * B_LOC]
        t_hi = trk_loc.astype(fp8).astype(np.float32)
        streams = {"th8": t_hi, "tl8": 16.0 * (trk_loc - t_hi),
                   "tq8": trk_loc / 4.0}
        # layout [(t,p), i, m] = stream[t*128 + m, i*128 + p]
        tmaps = {k: np.ascontiguousarray(
                     v.reshape(NT, 128, 2, 128).transpose(0, 3, 2, 1)
                 ).astype(fp8).reshape(NT * 128, 2, 128)
                 for k, v in streams.items()}
        in_maps.append({
            "ch8": staged.reshape(T_LOC // 2, 2 * SIZE),
            **tmaps,
            **shared,
        })

    return in_maps


# revision 81
# speedup vs baseline: 1.9170x; 1.0055x over previous
"""Trainium2 Bass kernel for the DependencyTreeLSTM node-reduction step.

Contract: kernel(**inputs) takes the FULL (unsharded) numpy inputs exactly as
produced by setup_inputs() and returns the FULL [B, 2*SIZE] float32 output.

Strategy (8 NeuronCores, data-parallel over the node axis, no collectives):
  - Each core owns B/8 = 2048 nodes (= 32768 children rows).
  - Only the h-half of `children` is needed in bulk (the c-half matters only
    for the first 16 rows, see below). It is staged **fp8 e4m3**: the
    children stream is ~2/3 of all HBM traffic and fp8 halves it; the
    per-node means average 16 children so the quantization noise largely
    cancels (measured ~9e-3 scale-relative max error, gate is 2e-2).
  - The children stream is split over the three DMA queues (SP-HWDGE,
    ACT-HWDGE, Pool-SWDGE), which transfer concurrently; each queue is
    internally FIFO so chunk order is hand-scheduled around need times.
  - Segment mean per 128-node tile, produced feature-major directly: 16
    DoubleRow fp8 matmuls with the children blocks stationary and a
    0/(1/16) selection strip moving -> PSUM holds mean^T (the /16 lives in
    the strip values), quantized to fp8 by one VectorE copy.
  - iou = [mean, tracking_h, 1] @ 16*[W_iou; W_iou_track; b_iou] runs on
    PE entirely as fp8 DoubleRow groups; the activation applies the 1/16.
    Tracking needs ~bf16 accuracy (plain fp8 fails the gate at 2.9e-2), so
    its GEMM is split into three fp8 rank-compensation groups with
    power-of-2 scale balancing: q(trk)@q(16W) + q(16(trk-q(trk)))@q(W) +
    q(trk/4)@q(4(16W-q(16W))) - measured equal to the bf16 path.
  - sigmoid/tanh on ScalarE, elementwise on VectorE, node-major DMA out.
    The PE instruction stream is the wall-clock floor (hwdecode makes its
    per-instruction overhead ~2 ns); DMA is hand-scheduled across the
    three queues so every input lands just before its consumer.
  - The reference computes fc_b = cumsum(fc)[lens-1]; with lens == 16
    everywhere this is one shared prefix over the first 16 rows of fc.
    Each core recomputes that tiny [1, 256] vector on device and
    broadcasts it with a K=1 ones outer-product matmul.

If the inputs do not match the structural assumptions (uniform 16-child
segments), we fall back to a plain numpy implementation of the reference
(never taken for the benchmark inputs).
"""

import sys

if "/opt/trn_rl_repo" not in sys.path:
    sys.path.insert(0, "/opt/trn_rl_repo")

import numpy as np

B = 16384
CH = 16
T = B * CH
SIZE = 256
TR = 256
NCORES = 8
B_LOC = B // NCORES          # 2048 nodes per core
T_LOC = B_LOC * CH           # 32768 children rows per core
NT = B_LOC // 128            # 16 node-tiles of 128 nodes per core
NJ = 8                       # DoubleRow matmuls per node-tile (256 rows each)
# children chunks: (queue, tiles). The sync HWDGE queue takes the earliest
# tiles (fast handoff); the SWDGE queue takes the bulk that is needed later.
CH_CHUNKS = [("sync", [0]), ("gpsimd", [1]), ("sync", [2, 3]),
             ("sync", [4, 5]), ("gpsimd", [6, 7]), ("gpsimd", [8, 9]),
             ("sync", [10, 11]), ("gpsimd", [12, 13]), ("sync", [14]),
             ("gpsimd", [15])]
# tile index -> chunk ids whose DMA fires at the top of that tile
CH_ISSUE = {0: [0, 1], 1: [2], 2: [3, 4], 5: [5], 7: [6], 8: [7], 9: [9],
            11: [8]}
TRK_SPLIT = 2                # tracking tiles on sync before the rest on pool
OUT_PLAN = [4, 4, 4, 3, 1]   # output DMA groups; small tail groups
Y_ISSUE = {6: 0, 10: 1, 12: 2, 14: 3, 15: 4}  # tile at which y-group fires

_cache = {}


def _sigmoid(x):
    return 1.0 / (1.0 + np.exp(-x))


def _reference_np(children, tracking, W_iou, b_iou, W_f, b_f, W_iou_track,
                  W_f_track, segment_ids, lens):
    size = W_f.shape[0]
    nb = tracking.shape[0]
    tr_h = tracking[:, : tracking.shape[1] // 2]
    sums = np.zeros((nb, children.shape[1]), np.float32)
    np.add.at(sums, segment_ids, children)
    mean_h = (sums / lens[:, None].astype(np.float32))[:, :size]
    iou = mean_h @ W_iou + b_iou + tr_h @ W_iou_track
    i, o, u = np.split(iou, 3, axis=1)
    i, o, u = _sigmoid(i), _sigmoid(o), np.tanh(u)
    f = children[:, :size] @ W_f + b_f + (tr_h @ W_f_track)[segment_ids]
    fc = _sigmoid(f) * children[:, size:]
    cs = np.cumsum(fc, axis=0, dtype=np.float32)
    fc_b = cs[lens - 1]
    c = i * u + fc_b
    h = o * c
    return np.concatenate([h, c], axis=1).astype(np.float32)


def _build_nc():
    import concourse.tile as tile
    from concourse import bacc, mybir

    f32 = mybir.dt.float32
    f32r = mybir.dt.float32r
    bf16 = mybir.dt.bfloat16
    fp8 = mybir.dt.float8e4
    SIG = mybir.ActivationFunctionType.Sigmoid
    TANH = mybir.ActivationFunctionType.Tanh
    COPY = mybir.ActivationFunctionType.Copy
    DR = mybir.MatmulPerfMode.DoubleRow

    nc = bacc.Bacc("TRN2", target_bir_lowering=False, debug=False,
                   num_devices=NCORES)

    # children h-half, fp8, staged (t, p, j, i*d) so each partition's DMA
    # line is one contiguous 4 KB run per node-tile
    ch8 = nc.declare_dram_parameter("ch8", [T_LOC // 2, 2 * SIZE], fp8,
                                    isOutput=False)
    # tracking h transposed, three fp8 streams: hi = q(trk),
    # lo = q(16*(trk - hi)), q4 = q(trk/4); with the matching W streams the
    # three DoubleRow groups reproduce trk @ (16*W_track) to ~bf16 accuracy
    th8 = nc.declare_dram_parameter("th8", [NT * 128, 2, 128], fp8,
                                    isOutput=False)
    tl8 = nc.declare_dram_parameter("tl8", [NT * 128, 2, 128], fp8,
                                    isOutput=False)
    tq8 = nc.declare_dram_parameter("tq8", [NT * 128, 2, 128], fp8,
                                    isOutput=False)
    # selection strip: strip[p, i, x] = 1/16 iff x == 8i + p//16 + 112;
    # columns [240:368) hold the DR-bias lhs (partition 0: half0=1, half1=0)
    sel8 = nc.declare_dram_parameter("sel8", [128, 2, 368], fp8, isOutput=False)
    # W_iou as a DoubleRow K-group: wdr[p, i, n] = W_iou[i*128+p, n]
    wdr = nc.declare_dram_parameter("wdr", [128, 2, 3 * SIZE], fp8,
                                    isOutput=False)
    # W_iou_track fp8 K-groups: hi = q(16W), raw = q(W),
    # res = q(4*(16W - hi))
    wt1 = nc.declare_dram_parameter("wt1", [128, 2, 3 * SIZE], fp8,
                                    isOutput=False)
    wt2 = nc.declare_dram_parameter("wt2", [128, 2, 3 * SIZE], fp8,
                                    isOutput=False)
    wt3 = nc.declare_dram_parameter("wt3", [128, 2, 3 * SIZE], fp8,
                                    isOutput=False)
    # bias as DR rhs: row 0 = 16*b_iou, row 1 = 0 (lhs supplies the 1/16)
    bias2 = nc.declare_dram_parameter("bias2", [1, 2, 3 * SIZE], fp8,
                                      isOutput=False)
    # fc prefix inputs packed into one tensor: block b<4 holds the K-block
    # [X^T | Wcat] pair, block 4 row 0 holds the [ones | b_f] tail
    pfx = nc.declare_dram_parameter("pfx", [128, 5, CH + SIZE], bf16,
                                    isOutput=False)
    chc16 = nc.declare_dram_parameter("chc16", [CH, SIZE], f32, isOutput=False)
    y = nc.declare_dram_parameter("y", [B_LOC, 2 * SIZE], bf16, isOutput=True)

    chv = ch8[:].rearrange("(t p j) q -> p t j q", p=128, j=NJ)  # q = 2*256
    thv = th8[:].rearrange("(t p) i m -> p t i m", p=128)
    tlv = tl8[:].rearrange("(t p) i m -> p t i m", p=128)
    tqv = tq8[:].rearrange("(t p) i m -> p t i m", p=128)
    yv = y[:].rearrange("(t p) d -> p t d", p=128)
    assert sorted(sum((ts for _, ts in CH_CHUNKS), [])) == list(range(NT))
    assert sum(OUT_PLAN) == NT

    with tile.TileContext(nc) as tc:
        with (
            tc.tile_pool(name="consts", bufs=1) as consts,
            tc.tile_pool(name="chpool_a", bufs=3) as chpool_a,
            tc.tile_pool(name="chpool_b", bufs=3) as chpool_b,
            tc.tile_pool(name="ztpool", bufs=3) as ztpool,
            tc.tile_pool(name="actpool", bufs=3) as actpool,
            tc.tile_pool(name="outpool", bufs=2) as outpool,
            tc.tile_pool(name="psum_s", bufs=2, space="PSUM") as psum_s,
            tc.tile_pool(name="psum_i", bufs=3, space="PSUM") as psum_i,
        ):
            # ---- DMA schedule ----
            # sync  : ch{0}, ch{1}, trk[0:4], ch{4,5}, y0, ch{8,9}, y1,
            #         ch{14}, y2, ch{10,11}... (children issued in-loop)
            # gpsimd: sel, ch{2,3}, trk[4:16], ch{6,7}, ch{12,13}, ch{15}
            # scalar: pfx, chc16, wdr, wtk, bias
            th_sb = consts.tile([128, NT, 2, 128], fp8)
            tl_sb = consts.tile([128, NT, 2, 128], fp8)
            tq_sb = consts.tile([128, NT, 2, 128], fp8)
            sel_sb = consts.tile([128, 2, 368], fp8)
            nc.gpsimd.dma_start(out=sel_sb, in_=sel8[:])
            wdr_sb = consts.tile([128, 2, 3 * SIZE], fp8)
            nc.scalar.dma_start(out=wdr_sb, in_=wdr[:])
            wt1_sb = consts.tile([128, 2, 3 * SIZE], fp8)
            nc.scalar.dma_start(out=wt1_sb, in_=wt1[:])
            wt2_sb = consts.tile([128, 2, 3 * SIZE], fp8)
            nc.scalar.dma_start(out=wt2_sb, in_=wt2[:])
            wt3_sb = consts.tile([128, 2, 3 * SIZE], fp8)
            nc.scalar.dma_start(out=wt3_sb, in_=wt3[:])
            bias_sb = consts.tile([1, 2, 3 * SIZE], fp8)
            nc.scalar.dma_start(out=bias_sb, in_=bias2[:])
            pfx_sb = consts.tile([128, 5, CH + SIZE], bf16)
            chc_sb = consts.tile([CH, SIZE], f32)
            blhs_sb = sel_sb[0:1, :, 240:368]
            ones_f = consts.tile([CH, 128], f32)
            nc.vector.memset(ones_f, 1.0)
            ones_v = ones_f[:].bitcast(f32r)
            ones1 = ones_v[0:1, :]
            ones16 = ones_v[:, 0:1]

            bc_sb = consts.tile([128, SIZE], bf16)
            psum_f_box = []

            def emit_prefix():
                # fc prefix: fc_b = sum_{t<16} sigmoid(X @ Wcat)[t] * ch_c[t]
                # Emitted between tile 0's mean and its iou: the PE fills the
                # window where it would otherwise stall on the wtk/bias DMAs.
                psum_f = psum_i.tile([CH, SIZE], f32, tag="pi")
                for b in range(4):
                    nc.tensor.matmul(psum_f, lhsT=pfx_sb[:, b, 0:CH],
                                     rhs=pfx_sb[:, b, CH:],
                                     start=(b == 0), stop=False)
                nc.tensor.matmul(psum_f, lhsT=pfx_sb[0:1, 4, 0:CH],
                                 rhs=pfx_sb[0:1, 4, CH:],
                                 start=False, stop=True)
                psum_f_box.append(psum_f)

            def emit_prefix_reduce():
                psum_f = psum_f_box.pop()
                sig_sb = consts.tile([CH, SIZE], f32)
                nc.scalar.activation(out=sig_sb, in_=psum_f, func=SIG)
                fc_sb = consts.tile([CH, SIZE], f32r)
                nc.vector.tensor_mul(fc_sb, sig_sb, chc_sb)
                psum_pref = psum_i.tile([1, SIZE], f32, tag="pi")
                nc.tensor.matmul(psum_pref, lhsT=ones16,
                                 rhs=fc_sb[:], start=True, stop=True)
                pref_sb = consts.tile([1, SIZE], f32r)
                nc.vector.tensor_copy(pref_sb, psum_pref)
                psum_bc = psum_i.tile([128, SIZE], f32, tag="pi")
                nc.tensor.matmul(psum_bc, lhsT=ones1,
                                 rhs=pref_sb[:], start=True, stop=True)
                nc.vector.tensor_copy(bc_sb, psum_bc)

            # ---- main loop over node-tiles ----
            chunk_of = {}
            for ci, (q, ts) in enumerate(CH_CHUNKS):
                for hi, t in enumerate(ts):
                    chunk_of[t] = (ci, hi)
            nmax = max(len(ts) for _, ts in CH_CHUNKS)
            ogrp_of = []
            for ui, n in enumerate(OUT_PLAN):
                ogrp_of += [(ui, hi, n) for hi in range(n)]
            ogrp_starts = [sum(OUT_PLAN[:ui]) for ui in range(len(OUT_PLAN))]
            ch_sbs = {}
            out_grps = {}
            zt_sbs = {}

            def stage_sums(t):
                # fire children chunk DMAs scheduled at this tile index
                for ci in CH_ISSUE.get(t, ()):
                    q, ts = CH_CHUNKS[ci]
                    pool = chpool_a if q == "sync" else chpool_b
                    ch_sbn = pool.tile([128, nmax, NJ, 2 * SIZE],
                                       fp8, name=f"ch{ci}", tag=f"ch_{q}")
                    dma_eng = {"sync": nc.sync, "gpsimd": nc.gpsimd}[q]
                    if ci == 0:
                        # split so the very first half-tile lands sooner
                        dma_eng.dma_start(out=ch_sbn[:, 0, 0:NJ // 2],
                                          in_=chv[:, 0, 0:NJ // 2])
                        dma_eng.dma_start(out=ch_sbn[:, 0, NJ // 2:],
                                          in_=chv[:, 0, NJ // 2:])
                    else:
                        dma_eng.dma_start(out=ch_sbn[:, :len(ts)],
                                          in_=chv[:, ts[0]:ts[0] + len(ts)])
                    ch_sbs[ci] = ch_sbn
                # prefix consts + tracking ride the sync queue early
                if t == 0:
                    for sb, v in ((th_sb, thv), (tl_sb, tlv), (tq_sb, tqv)):
                        nc.sync.dma_start(out=sb[:, 0:TRK_SPLIT],
                                          in_=v[:, 0:TRK_SPLIT])
                    nc.sync.dma_start(out=pfx_sb, in_=pfx[:])
                    nc.sync.dma_start(out=chc_sb, in_=chc16[:])
                elif t == 1:
                    for sb, v in ((th_sb, thv), (tl_sb, tlv), (tq_sb, tqv)):
                        nc.gpsimd.dma_start(out=sb[:, TRK_SPLIT:NT],
                                            in_=v[:, TRK_SPLIT:NT])
                ci, hh = chunk_of[t]
                ch_sb = ch_sbs[ci][:, hh]          # [128, NJ, 512]

                # segment mean, produced feature-major directly: children
                # blocks stationary, selection strip (values 1/16) moving.
                # out[d, node] = sum over the block's 256 child rows; 16
                # DoubleRow fp8 matmuls accumulate all 2048 rows.
                psum_sum = psum_s.tile([128, 2, 128], f32, name=f"ps{t}",
                                       tag="ps")
                for j in range(NJ):
                    chj = ch_sb[:, j].rearrange("p (i d) -> p i d", i=2)
                    selj = sel_sb[:, :, 112 - 16 * j:240 - 16 * j]
                    for h in range(2):
                        nc.tensor.matmul(psum_sum[:, h],
                                         lhsT=chj[:, :, 128 * h:128 * h + 128],
                                         rhs=selj,
                                         start=(j == 0 and h == 0),
                                         stop=(j == NJ - 1 and h == 1),
                                         perf_mode=DR)
                zt_sb = ztpool.tile([128, 2, 128], fp8, name=f"zt{t}", tag="zt")
                nc.vector.tensor_copy(zt_sb, psum_sum)
                zt_sbs[t] = zt_sb
                if t == 1:
                    emit_prefix()

            def stage_rest(t):
                zt_sb = zt_sbs.pop(t)
                # iou[node, 0:768] = mean @ W_iou + trk @ W_iou_track + b_iou
                # PSUM groups are per 2KB zero-region (512 f32 cols): one
                # start and one stop per region; DR moving capped at 256 cols
                psum_iou = psum_i.tile([128, 3 * SIZE], f32, name=f"pi{t}",
                                       tag="pi")
                for ck in range(3):
                    ds = slice(256 * ck, 256 * ck + 256)
                    first = ck in (0, 2)
                    last = ck in (1, 2)
                    for gi, (lhs, rhs) in enumerate((
                            (th_sb[:, t], wt1_sb), (tl_sb[:, t], wt2_sb),
                            (tq_sb[:, t], wt3_sb), (zt_sb, wdr_sb),
                            (blhs_sb, bias_sb))):
                        nc.tensor.matmul(psum_iou[:, ds], lhsT=lhs,
                                         rhs=rhs[:, :, ds],
                                         start=(first and gi == 0),
                                         stop=(last and gi == 4),
                                         perf_mode=DR)

                if t == 0:
                    emit_prefix_reduce()
                # PSUM holds 16x iou (weights staged pre-scaled by 16 so the
                # fp8 W_iou values stay clear of the subnormal range)
                act_sb = actpool.tile([128, 3 * SIZE], bf16, name=f"ac{t}",
                                      tag="ac")
                nc.scalar.activation(out=act_sb[:, 0:512],
                                     in_=psum_iou[:, 0:512], func=SIG,
                                     scale=1.0 / 16.0)
                nc.scalar.activation(out=act_sb[:, 512:768],
                                     in_=psum_iou[:, 512:768], func=TANH,
                                     scale=1.0 / 16.0)

                u, gh, gn = ogrp_of[t]
                if gh == 0:
                    out_grps[u] = outpool.tile([128, max(OUT_PLAN), 2 * SIZE],
                                               bf16, name=f"ot{u}", tag="ot")
                out_sb = out_grps[u][:, gh]
                # c = i*u + fc_b ; h = o*c
                nc.vector.tensor_mul(out_sb[:, 256:512], act_sb[:, 0:256],
                                     act_sb[:, 512:768])
                nc.vector.tensor_add(out_sb[:, 256:512], out_sb[:, 256:512],
                                     bc_sb)
                nc.vector.tensor_mul(out_sb[:, 0:256], act_sb[:, 256:512],
                                     out_sb[:, 256:512])
                # y groups ride the sync queue, issued late so they never
                # block a children chunk the compute still needs
                if t in Y_ISSUE:
                    u2 = Y_ISSUE[t]
                    g0 = ogrp_starts[u2]
                    gn2 = OUT_PLAN[u2]
                    # the last group rides the (idle) scalar queue so it
                    # does not wait behind the previous group's transfer
                    eng = nc.scalar if u2 == len(OUT_PLAN) - 2 else nc.sync
                    eng.dma_start(out=yv[:, g0:g0 + gn2],
                                  in_=out_grps[u2][:, :gn2])

            # software pipeline: sums run one tile ahead of iou/act/ew so PE
            # fills the initial weight-DMA wait with the next tile's sums
            for t in range(NT + 1):
                if t < NT:
                    stage_sums(t)
                if t >= 1:
                    stage_rest(t - 1)

    nc.finalize()
    return nc


def _get_nc():
    if "nc" not in _cache:
        _cache["nc"] = _build_nc()
    return _cache["nc"]


def kernel(**inputs):
    children = np.ascontiguousarray(np.asarray(inputs["children"], np.float32))
    tracking = np.ascontiguousarray(np.asarray(inputs["tracking"], np.float32))
    W_iou = np.asarray(inputs["W_iou"], np.float32)
    b_iou = np.asarray(inputs["b_iou"], np.float32)
    W_f = np.asarray(inputs["W_f"], np.float32)
    b_f = np.asarray(inputs["b_f"], np.float32)
    W_iou_track = np.asarray(inputs["W_iou_track"], np.float32)
    W_f_track = np.asarray(inputs["W_f_track"], np.float32)
    segment_ids = np.asarray(inputs["segment_ids"], np.int32)
    lens = np.asarray(inputs["lens"], np.int32)

    structured = (
        children.shape == (T, 2 * SIZE)
        and tracking.shape == (B, 2 * TR)
        and W_iou.shape == (SIZE, 3 * SIZE)
        and W_f.shape == (SIZE, SIZE)
        and W_iou_track.shape == (TR, 3 * SIZE)
        and W_f_track.shape == (TR, SIZE)
        and lens.shape == (B,)
        and segment_ids.shape == (T,)
        and bool((lens == CH).all())
        and bool((segment_ids == np.repeat(np.arange(B, dtype=np.int32), CH)).all())
    )
    if not structured:
        return _reference_np(children, tracking, W_iou, b_iou, W_f, b_f,
                             W_iou_track, W_f_track, segment_ids, lens)

    from concourse.bass_utils import run_bass_kernel_spmd

    nc = _get_nc()
    in_maps = _stage_in_maps(children, tracking, W_iou, b_iou, W_f, b_f,
                             W_iou_track, W_f_track, segment_ids)

    res = run_bass_kernel_spmd(nc, in_maps, core_ids=list(range(NCORES)))
    _cache["last_exec_time_ns"] = res.exec_time_ns
    out = np.concatenate([np.asarray(r["y"]).astype(np.float32)
                          for r in res.results], axis=0)
    return out


def _stage_in_maps(children, tracking, W_iou, b_iou, W_f, b_f,
                   W_iou_track, W_f_track, segment_ids):
    import ml_dtypes

    bf16 = ml_dtypes.bfloat16
    fp8 = ml_dtypes.float8_e4m3
    tr_h = tracking[:, :TR]

    # selection strip: strip[p, i, x] = 1/16 iff x == 8i + p//16 + 112, so
    # the slice strip[:, :, 112-16j : 240-16j] selects node 16j+8i+p//16
    p = np.arange(128)
    sel = np.zeros((128, 2, 368), np.float32)
    for i in range(2):
        sel[p, i, 8 * i + p // 16 + 112] = 1.0 / 16.0
    sel[0, 0, 240:368] = 1.0

    # weights pre-scaled by 16 (activation applies 1/16): keeps the fp8
    # W_iou values in the e4m3 normal range
    def kblocks(w):
        return np.ascontiguousarray(
            w.reshape(2, 128, 3 * SIZE).transpose(1, 0, 2)).astype(fp8)

    wdr = kblocks(16.0 * W_iou)
    WS = 16.0 * W_iou_track
    WS_hi = WS.astype(fp8).astype(np.float32)
    wt1 = kblocks(WS)
    wt2 = kblocks(W_iou_track)
    wt3 = kblocks(4.0 * (WS - WS_hi))
    bias2 = np.zeros((1, 2, 3 * SIZE), np.float32)
    bias2[0, 0] = 16.0 * b_iou

    # prefix-f inputs: X = [ch_h[0:16], trk_h[seg[0:16]], 1],
    # W = [W_f; W_f_track; b_f], packed K-blocks [X^T | Wcat]
    X = np.concatenate([
        children[:CH, :SIZE],
        tr_h[segment_ids[:CH]],
        np.ones((CH, 1), np.float32),
    ], axis=1)                                       # [16, 513]
    XT = X.T                                         # [513, 16]
    WC = np.concatenate([W_f, W_f_track], axis=0)    # [512, 256]
    pfx = np.zeros((128, 5, CH + SIZE), np.float32)
    for b in range(4):
        pfx[:, b, :CH] = XT[b * 128:(b + 1) * 128]
        pfx[:, b, CH:] = WC[b * 128:(b + 1) * 128]
    pfx[0, 4, :CH] = 1.0
    pfx[0, 4, CH:] = b_f
    chc16 = np.ascontiguousarray(children[:CH, SIZE:])

    shared = {"sel8": sel.astype(fp8), "wdr": wdr, "wt1": wt1, "wt2": wt2,
              "wt3": wt3, "bias2": bias2.astype(fp8),
              "pfx": pfx.astype(bf16), "chc16": chc16}
    in_maps = []
    for c in range(NCORES):
        shard = children[c * T_LOC:(c + 1) * T_LOC, :SIZE].astype(fp8)
        # staged[t, p, j, i, d] = shard[t*2048 + j*256 + i*128 + p, d]
        staged = np.ascontiguousarray(
            shard.reshape(NT, NJ, 2, 128, SIZE).transpose(0, 3, 1, 2, 4))
        trk_loc = tr_h[c * B_LOC:(c + 1) * B_LOC]
        t_hi = trk_loc.astype(fp8).astype(np.float32)
        streams = {"th8": t_hi, "tl8": 16.0 * (trk_loc - t_hi),
                   "tq8": trk_loc / 4.0}
        # layout [(t,p), i, m] = stream[t*128 + m, i*128 + p]
        tmaps = {k: np.ascontiguousarray(
                     v.reshape(NT, 128, 2, 128).transpose(0, 3, 2, 1)
                 ).astype(fp8).reshape(NT * 128, 2, 128)
                 for k, v in streams.items()}
        in_maps.append({
            "ch8": staged.reshape(T_LOC // 2, 2 * SIZE),
            **tmaps,
            **shared,
        })

    return in_maps


# revision 95
# speedup vs baseline: 1.9489x; 1.0166x over previous
"""Trainium2 Bass kernel for the DependencyTreeLSTM node-reduction step.

Contract: kernel(**inputs) takes the FULL (unsharded) numpy inputs exactly as
produced by setup_inputs() and returns the FULL [B, 2*SIZE] float32 output.

Strategy (8 NeuronCores, data-parallel over the node axis, no collectives):
  - Each core owns B/8 = 2048 nodes (= 32768 children rows).
  - Only the h-half of `children` is needed in bulk (the c-half matters only
    for the first 16 rows, see below). It is staged **fp8 e4m3**: the
    children stream is ~2/3 of all HBM traffic and fp8 halves it; the
    per-node means average 16 children so the quantization noise largely
    cancels (measured ~9e-3 scale-relative max error, gate is 2e-2).
  - The children stream is split over the three DMA queues (SP-HWDGE,
    ACT-HWDGE, Pool-SWDGE), which transfer concurrently; each queue is
    internally FIFO so chunk order is hand-scheduled around need times.
  - Segment mean per 128-node tile, produced feature-major directly: 16
    DoubleRow fp8 matmuls with the children blocks stationary and a
    0/(1/16) selection strip moving -> PSUM holds mean^T (the /16 lives in
    the strip values), quantized to fp8 by one VectorE copy.
  - iou = [mean, tracking_h, 1] @ 16*[W_iou; W_iou_track; b_iou] runs on
    PE entirely as fp8 DoubleRow groups; the activation applies the 1/16.
    Tracking needs ~bf16 accuracy (plain fp8 fails the gate at 2.9e-2), so
    its GEMM is split into three fp8 rank-compensation groups with
    power-of-2 scale balancing: q(trk)@q(16W) + q(16(trk-q(trk)))@q(W) +
    q(trk)@q(16W-q(16W)) - measured equal to the bf16 path, and only two
    tracking streams ship over HBM (the hi stream is reused).
  - sigmoid/tanh on ScalarE, elementwise on VectorE, node-major DMA out.
    The PE instruction stream is the wall-clock floor (hwdecode makes its
    per-instruction overhead ~2 ns); DMA is hand-scheduled across the
    three queues so every input lands just before its consumer.
  - The reference computes fc_b = cumsum(fc)[lens-1]; with lens == 16
    everywhere this is one shared prefix over the first 16 rows of fc.
    Each core recomputes that tiny [1, 256] vector on device and
    broadcasts it with a K=1 ones outer-product matmul.

If the inputs do not match the structural assumptions (uniform 16-child
segments), we fall back to a plain numpy implementation of the reference
(never taken for the benchmark inputs).
"""

import sys

if "/opt/trn_rl_repo" not in sys.path:
    sys.path.insert(0, "/opt/trn_rl_repo")

import numpy as np

B = 16384
CH = 16
T = B * CH
SIZE = 256
TR = 256
NCORES = 8
B_LOC = B // NCORES          # 2048 nodes per core
T_LOC = B_LOC * CH           # 32768 children rows per core
NT = B_LOC // 128            # 16 node-tiles of 128 nodes per core
NJ = 8                       # DoubleRow matmuls per node-tile (256 rows each)
# children chunks: (queue, tiles). The sync HWDGE queue takes the earliest
# tiles (fast handoff); the SWDGE queue takes the bulk that is needed later.
CH_CHUNKS = [("sync", [0]), ("gpsimd", [1]), ("sync", [2, 3]),
             ("sync", [4, 5]), ("gpsimd", [6, 7]), ("gpsimd", [8, 9]),
             ("sync", [10, 11]), ("gpsimd", [12, 13]), ("sync", [14]),
             ("gpsimd", [15])]
# tile index -> chunk ids whose DMA fires at the top of that tile
CH_ISSUE = {0: [0, 1], 1: [2], 2: [3, 4], 5: [5], 7: [6], 8: [7], 9: [9],
            11: [8]}
TRK_SPLIT = 3                # tracking tiles on sync before the rest on pool
OUT_PLAN = [4, 4, 4, 3, 1]   # output DMA groups; small tail groups
Y_ISSUE = {6: 0, 10: 1, 12: 2, 14: 3, 15: 4}  # tile at which y-group fires

_cache = {}


def _sigmoid(x):
    return 1.0 / (1.0 + np.exp(-x))


def _reference_np(children, tracking, W_iou, b_iou, W_f, b_f, W_iou_track,
                  W_f_track, segment_ids, lens):
    size = W_f.shape[0]
    nb = tracking.shape[0]
    tr_h = tracking[:, : tracking.shape[1] // 2]
    sums = np.zeros((nb, children.shape[1]), np.float32)
    np.add.at(sums, segment_ids, children)
    mean_h = (sums / lens[:, None].astype(np.float32))[:, :size]
    iou = mean_h @ W_iou + b_iou + tr_h @ W_iou_track
    i, o, u = np.split(iou, 3, axis=1)
    i, o, u = _sigmoid(i), _sigmoid(o), np.tanh(u)
    f = children[:, :size] @ W_f + b_f + (tr_h @ W_f_track)[segment_ids]
    fc = _sigmoid(f) * children[:, size:]
    cs = np.cumsum(fc, axis=0, dtype=np.float32)
    fc_b = cs[lens - 1]
    c = i * u + fc_b
    h = o * c
    return np.concatenate([h, c], axis=1).astype(np.float32)


def _build_nc():
    import concourse.tile as tile
    from concourse import bacc, mybir

    f32 = mybir.dt.float32
    f32r = mybir.dt.float32r
    bf16 = mybir.dt.bfloat16
    fp8 = mybir.dt.float8e4
    SIG = mybir.ActivationFunctionType.Sigmoid
    TANH = mybir.ActivationFunctionType.Tanh
    COPY = mybir.ActivationFunctionType.Copy
    DR = mybir.MatmulPerfMode.DoubleRow

    nc = bacc.Bacc("TRN2", target_bir_lowering=False, debug=False,
                   num_devices=NCORES)

    # children h-half, fp8, staged (t, p, j, i*d) so each partition's DMA
    # line is one contiguous 4 KB run per node-tile
    ch8 = nc.declare_dram_parameter("ch8", [T_LOC // 2, 2 * SIZE], fp8,
                                    isOutput=False)
    # tracking h transposed, two fp8 streams: hi = q(trk) and
    # lo = q(16*(trk - hi)); with the matching W streams the three DoubleRow
    # groups (hi reused for the W-residual group) reproduce
    # trk @ (16*W_track) to ~bf16 accuracy
    th8 = nc.declare_dram_parameter("th8", [NT * 128, 2, 128], fp8,
                                    isOutput=False)
    tl8 = nc.declare_dram_parameter("tl8", [NT * 128, 2, 128], fp8,
                                    isOutput=False)
    # selection strip: strip[p, i, x] = 1/16 iff x == 8i + p//16 + 112;
    # columns [240:368) hold the DR-bias lhs (partition 0: half0=1, half1=0)
    sel8 = nc.declare_dram_parameter("sel8", [128, 2, 368], fp8, isOutput=False)
    # W_iou as a DoubleRow K-group: wdr[p, i, n] = W_iou[i*128+p, n]
    wdr = nc.declare_dram_parameter("wdr", [128, 2, 3 * SIZE], fp8,
                                    isOutput=False)
    # W_iou_track fp8 K-groups: hi = q(16W), raw = q(W), res = q(16W - hi)
    wt1 = nc.declare_dram_parameter("wt1", [128, 2, 3 * SIZE], fp8,
                                    isOutput=False)
    wt2 = nc.declare_dram_parameter("wt2", [128, 2, 3 * SIZE], fp8,
                                    isOutput=False)
    wt3 = nc.declare_dram_parameter("wt3", [128, 2, 3 * SIZE], fp8,
                                    isOutput=False)
    # bias as DR rhs: row 0 = 16*b_iou, row 1 = 0 (lhs supplies the 1/16)
    bias2 = nc.declare_dram_parameter("bias2", [1, 2, 3 * SIZE], fp8,
                                      isOutput=False)
    # fc prefix inputs packed into one tensor: block b<4 holds the K-block
    # [X^T | Wcat] pair, block 4 row 0 holds the [ones | b_f] tail
    pfx = nc.declare_dram_parameter("pfx", [128, 5, CH + SIZE], bf16,
                                    isOutput=False)
    chc16 = nc.declare_dram_parameter("chc16", [CH, SIZE], f32, isOutput=False)
    y = nc.declare_dram_parameter("y", [B_LOC, 2 * SIZE], bf16, isOutput=True)

    chv = ch8[:].rearrange("(t p j) q -> p t j q", p=128, j=NJ)  # q = 2*256
    thv = th8[:].rearrange("(t p) i m -> p t i m", p=128)
    tlv = tl8[:].rearrange("(t p) i m -> p t i m", p=128)
    yv = y[:].rearrange("(t p) d -> p t d", p=128)
    assert sorted(sum((ts for _, ts in CH_CHUNKS), [])) == list(range(NT))
    assert sum(OUT_PLAN) == NT

    with tile.TileContext(nc) as tc:
        with (
            tc.tile_pool(name="consts", bufs=1) as consts,
            tc.tile_pool(name="chpool_a", bufs=3) as chpool_a,
            tc.tile_pool(name="chpool_b", bufs=3) as chpool_b,
            tc.tile_pool(name="ztpool", bufs=3) as ztpool,
            tc.tile_pool(name="actpool", bufs=3) as actpool,
            tc.tile_pool(name="outpool", bufs=2) as outpool,
            tc.tile_pool(name="psum_s", bufs=2, space="PSUM") as psum_s,
            tc.tile_pool(name="psum_i", bufs=3, space="PSUM") as psum_i,
        ):
            # ---- DMA schedule ----
            # sync  : ch{0}, ch{1}, trk[0:4], ch{4,5}, y0, ch{8,9}, y1,
            #         ch{14}, y2, ch{10,11}... (children issued in-loop)
            # gpsimd: sel, ch{2,3}, trk[4:16], ch{6,7}, ch{12,13}, ch{15}
            # scalar: pfx, chc16, wdr, wtk, bias
            th_sb = consts.tile([128, NT, 2, 128], fp8)
            tl_sb = consts.tile([128, NT, 2, 128], fp8)
            sel_sb = consts.tile([128, 2, 368], fp8)
            nc.gpsimd.dma_start(out=sel_sb, in_=sel8[:])
            wdr_sb = consts.tile([128, 2, 3 * SIZE], fp8)
            nc.scalar.dma_start(out=wdr_sb, in_=wdr[:])
            wt1_sb = consts.tile([128, 2, 3 * SIZE], fp8)
            nc.scalar.dma_start(out=wt1_sb, in_=wt1[:])
            wt2_sb = consts.tile([128, 2, 3 * SIZE], fp8)
            nc.scalar.dma_start(out=wt2_sb, in_=wt2[:])
            wt3_sb = consts.tile([128, 2, 3 * SIZE], fp8)
            nc.scalar.dma_start(out=wt3_sb, in_=wt3[:])
            bias_sb = consts.tile([1, 2, 3 * SIZE], fp8)
            nc.scalar.dma_start(out=bias_sb, in_=bias2[:])
            pfx_sb = consts.tile([128, 5, CH + SIZE], bf16)
            chc_sb = consts.tile([CH, SIZE], f32)
            blhs_sb = sel_sb[0:1, :, 240:368]
            ones_f = consts.tile([CH, 128], f32)
            nc.vector.memset(ones_f, 1.0)
            ones_v = ones_f[:].bitcast(f32r)
            ones1 = ones_v[0:1, :]
            ones16 = ones_v[:, 0:1]

            bc_sb = consts.tile([128, SIZE], bf16)
            psum_f_box = []

            def emit_prefix():
                # fc prefix: fc_b = sum_{t<16} sigmoid(X @ Wcat)[t] * ch_c[t]
                # Emitted between tile 0's mean and its iou: the PE fills the
                # window where it would otherwise stall on the wtk/bias DMAs.
                psum_f = psum_i.tile([CH, SIZE], f32, tag="pi")
                for b in range(4):
                    nc.tensor.matmul(psum_f, lhsT=pfx_sb[:, b, 0:CH],
                                     rhs=pfx_sb[:, b, CH:],
                                     start=(b == 0), stop=False)
                nc.tensor.matmul(psum_f, lhsT=pfx_sb[0:1, 4, 0:CH],
                                 rhs=pfx_sb[0:1, 4, CH:],
                                 start=False, stop=True)
                psum_f_box.append(psum_f)

            def emit_prefix_reduce():
                psum_f = psum_f_box.pop()
                sig_sb = consts.tile([CH, SIZE], f32)
                nc.scalar.activation(out=sig_sb, in_=psum_f, func=SIG)
                fc_sb = consts.tile([CH, SIZE], f32r)
                nc.vector.tensor_mul(fc_sb, sig_sb, chc_sb)
                psum_pref = psum_i.tile([1, SIZE], f32, tag="pi")
                nc.tensor.matmul(psum_pref, lhsT=ones16,
                                 rhs=fc_sb[:], start=True, stop=True)
                pref_sb = consts.tile([1, SIZE], f32r)
                nc.vector.tensor_copy(pref_sb, psum_pref)
                psum_bc = psum_i.tile([128, SIZE], f32, tag="pi")
                nc.tensor.matmul(psum_bc, lhsT=ones1,
                                 rhs=pref_sb[:], start=True, stop=True)
                nc.vector.tensor_copy(bc_sb, psum_bc)

            # ---- main loop over node-tiles ----
            chunk_of = {}
            for ci, (q, ts) in enumerate(CH_CHUNKS):
                for hi, t in enumerate(ts):
                    chunk_of[t] = (ci, hi)
            nmax = max(len(ts) for _, ts in CH_CHUNKS)
            ogrp_of = []
            for ui, n in enumerate(OUT_PLAN):
                ogrp_of += [(ui, hi, n) for hi in range(n)]
            ogrp_starts = [sum(OUT_PLAN[:ui]) for ui in range(len(OUT_PLAN))]
            ch_sbs = {}
            out_grps = {}
            zt_sbs = {}

            def stage_sums(t):
                # fire children chunk DMAs scheduled at this tile index
                for ci in CH_ISSUE.get(t, ()):
                    q, ts = CH_CHUNKS[ci]
                    pool = chpool_a if q == "sync" else chpool_b
                    ch_sbn = pool.tile([128, nmax, NJ, 2 * SIZE],
                                       fp8, name=f"ch{ci}", tag=f"ch_{q}")
                    dma_eng = {"sync": nc.sync, "gpsimd": nc.gpsimd}[q]
                    if ci == 0:
                        # split so the very first half-tile lands sooner
                        dma_eng.dma_start(out=ch_sbn[:, 0, 0:NJ // 2],
                                          in_=chv[:, 0, 0:NJ // 2])
                        dma_eng.dma_start(out=ch_sbn[:, 0, NJ // 2:],
                                          in_=chv[:, 0, NJ // 2:])
                    else:
                        dma_eng.dma_start(out=ch_sbn[:, :len(ts)],
                                          in_=chv[:, ts[0]:ts[0] + len(ts)])
                    ch_sbs[ci] = ch_sbn
                # prefix consts + tracking ride the sync queue early
                if t == 0:
                    for sb, v in ((th_sb, thv), (tl_sb, tlv)):
                        nc.sync.dma_start(out=sb[:, 0:TRK_SPLIT],
                                          in_=v[:, 0:TRK_SPLIT])
                    nc.sync.dma_start(out=pfx_sb, in_=pfx[:])
                    nc.sync.dma_start(out=chc_sb, in_=chc16[:])
                elif t == 1:
                    for sb, v in ((th_sb, thv), (tl_sb, tlv)):
                        nc.gpsimd.dma_start(out=sb[:, TRK_SPLIT:NT],
                                            in_=v[:, TRK_SPLIT:NT])
                ci, hh = chunk_of[t]
                ch_sb = ch_sbs[ci][:, hh]          # [128, NJ, 512]

                # segment mean, produced feature-major directly: children
                # blocks stationary, selection strip (values 1/16) moving.
                # out[d, node] = sum over the block's 256 child rows; 16
                # DoubleRow fp8 matmuls accumulate all 2048 rows.
                psum_sum = psum_s.tile([128, 2, 128], f32, name=f"ps{t}",
                                       tag="ps")
                for j in range(NJ):
                    chj = ch_sb[:, j].rearrange("p (i d) -> p i d", i=2)
                    selj = sel_sb[:, :, 112 - 16 * j:240 - 16 * j]
                    for h in range(2):
                        nc.tensor.matmul(psum_sum[:, h],
                                         lhsT=chj[:, :, 128 * h:128 * h + 128],
                                         rhs=selj,
                                         start=(j == 0 and h == 0),
                                         stop=(j == NJ - 1 and h == 1),
                                         perf_mode=DR)
                zt_sb = ztpool.tile([128, 2, 128], fp8, name=f"zt{t}", tag="zt")
                nc.vector.tensor_copy(zt_sb, psum_sum)
                zt_sbs[t] = zt_sb
                if t == 1:
                    emit_prefix()

            def stage_rest(t):
                zt_sb = zt_sbs.pop(t)
                # iou[node, 0:768] = mean @ W_iou + trk @ W_iou_track + b_iou
                # PSUM groups are per 2KB zero-region (512 f32 cols): one
                # start and one stop per region; DR moving capped at 256 cols
                psum_iou = psum_i.tile([128, 3 * SIZE], f32, name=f"pi{t}",
                                       tag="pi")
                for ck in range(3):
                    ds = slice(256 * ck, 256 * ck + 256)
                    first = ck in (0, 2)
                    last = ck in (1, 2)
                    for gi, (lhs, rhs) in enumerate((
                            (th_sb[:, t], wt1_sb), (tl_sb[:, t], wt2_sb),
                            (th_sb[:, t], wt3_sb), (zt_sb, wdr_sb),
                            (blhs_sb, bias_sb))):
                        nc.tensor.matmul(psum_iou[:, ds], lhsT=lhs,
                                         rhs=rhs[:, :, ds],
                                         start=(first and gi == 0),
                                         stop=(last and gi == 4),
                                         perf_mode=DR)

                if t == 0:
                    emit_prefix_reduce()
                # PSUM holds 16x iou (weights staged pre-scaled by 16 so the
                # fp8 W_iou values stay clear of the subnormal range)
                act_sb = actpool.tile([128, 3 * SIZE], bf16, name=f"ac{t}",
                                      tag="ac")
                nc.scalar.activation(out=act_sb[:, 0:512],
                                     in_=psum_iou[:, 0:512], func=SIG,
                                     scale=1.0 / 16.0)
                nc.scalar.activation(out=act_sb[:, 512:768],
                                     in_=psum_iou[:, 512:768], func=TANH,
                                     scale=1.0 / 16.0)

                u, gh, gn = ogrp_of[t]
                if gh == 0:
                    out_grps[u] = outpool.tile([128, max(OUT_PLAN), 2 * SIZE],
                                               bf16, name=f"ot{u}", tag="ot")
                out_sb = out_grps[u][:, gh]
                # c = i*u + fc_b ; h = o*c
                nc.vector.tensor_mul(out_sb[:, 256:512], act_sb[:, 0:256],
                                     act_sb[:, 512:768])
                nc.vector.tensor_add(out_sb[:, 256:512], out_sb[:, 256:512],
                                     bc_sb)
                nc.vector.tensor_mul(out_sb[:, 0:256], act_sb[:, 256:512],
                                     out_sb[:, 256:512])
                # y groups ride the sync queue, issued late so they never
                # block a children chunk the compute still needs
                if t in Y_ISSUE:
                    u2 = Y_ISSUE[t]
                    g0 = ogrp_starts[u2]
                    gn2 = OUT_PLAN[u2]
                    # the last group rides the (idle) scalar queue so it
                    # does not wait behind the previous group's transfer
                    eng = nc.scalar if u2 == len(OUT_PLAN) - 2 else nc.sync
                    eng.dma_start(out=yv[:, g0:g0 + gn2],
                                  in_=out_grps[u2][:, :gn2])

            # software pipeline: sums run one tile ahead of iou/act/ew so PE
            # fills the initial weight-DMA wait with the next tile's sums
            for t in range(NT + 1):
                if t < NT:
                    stage_sums(t)
                if t >= 1:
                    stage_rest(t - 1)

    nc.finalize()
    return nc


def _get_nc():
    if "nc" not in _cache:
        _cache["nc"] = _build_nc()
    return _cache["nc"]


def kernel(**inputs):
    children = np.ascontiguousarray(np.asarray(inputs["children"], np.float32))
    tracking = np.ascontiguousarray(np.asarray(inputs["tracking"], np.float32))
    W_iou = np.asarray(inputs["W_iou"], np.float32)
    b_iou = np.asarray(inputs["b_iou"], np.float32)
    W_f = np.asarray(inputs["W_f"], np.float32)
    b_f = np.asarray(inputs["b_f"], np.float32)
    W_iou_track = np.asarray(inputs["W_iou_track"], np.float32)
    W_f_track = np.asarray(inputs["W_f_track"], np.float32)
    segment_ids = np.asarray(inputs["segment_ids"], np.int32)
    lens = np.asarray(inputs["lens"], np.int32)

    structured = (
        children.shape == (T, 2 * SIZE)
        and tracking.shape == (B, 2 * TR)
        and W_iou.shape == (SIZE, 3 * SIZE)
        and W_f.shape == (SIZE, SIZE)
        and W_iou_track.shape == (TR, 3 * SIZE)
        and W_f_track.shape == (TR, SIZE)
        and lens.shape == (B,)
        and segment_ids.shape == (T,)
        and bool((lens == CH).all())
        and bool((segment_ids == np.repeat(np.arange(B, dtype=np.int32), CH)).all())
    )
    if not structured:
        return _reference_np(children, tracking, W_iou, b_iou, W_f, b_f,
                             W_iou_track, W_f_track, segment_ids, lens)

    from concourse.bass_utils import run_bass_kernel_spmd

    nc = _get_nc()
    in_maps = _stage_in_maps(children, tracking, W_iou, b_iou, W_f, b_f,
                             W_iou_track, W_f_track, segment_ids)

    res = run_bass_kernel_spmd(nc, in_maps, core_ids=list(range(NCORES)))
    _cache["last_exec_time_ns"] = res.exec_time_ns
    out = np.concatenate([np.asarray(r["y"]).astype(np.float32)
                          for r in res.results], axis=0)
    return out


def _stage_in_maps(children, tracking, W_iou, b_iou, W_f, b_f,
                   W_iou_track, W_f_track, segment_ids):
    import ml_dtypes

    bf16 = ml_dtypes.bfloat16
    fp8 = ml_dtypes.float8_e4m3
    tr_h = tracking[:, :TR]

    # selection strip: strip[p, i, x] = 1/16 iff x == 8i + p//16 + 112, so
    # the slice strip[:, :, 112-16j : 240-16j] selects node 16j+8i+p//16
    p = np.arange(128)
    sel = np.zeros((128, 2, 368), np.float32)
    for i in range(2):
        sel[p, i, 8 * i + p // 16 + 112] = 1.0 / 16.0
    sel[0, 0, 240:368] = 1.0

    # weights pre-scaled by 16 (activation applies 1/16): keeps the fp8
    # W_iou values in the e4m3 normal range
    def kblocks(w):
        return np.ascontiguousarray(
            w.reshape(2, 128, 3 * SIZE).transpose(1, 0, 2)).astype(fp8)

    wdr = kblocks(16.0 * W_iou)
    WS = 16.0 * W_iou_track
    WS_hi = WS.astype(fp8).astype(np.float32)
    wt1 = kblocks(WS)
    wt2 = kblocks(W_iou_track)
    wt3 = kblocks(WS - WS_hi)
    bias2 = np.zeros((1, 2, 3 * SIZE), np.float32)
    bias2[0, 0] = 16.0 * b_iou

    # prefix-f inputs: X = [ch_h[0:16], trk_h[seg[0:16]], 1],
    # W = [W_f; W_f_track; b_f], packed K-blocks [X^T | Wcat]
    X = np.concatenate([
        children[:CH, :SIZE],
        tr_h[segment_ids[:CH]],
        np.ones((CH, 1), np.float32),
    ], axis=1)                                       # [16, 513]
    XT = X.T                                         # [513, 16]
    WC = np.concatenate([W_f, W_f_track], axis=0)    # [512, 256]
    pfx = np.zeros((128, 5, CH + SIZE), np.float32)
    for b in range(4):
        pfx[:, b, :CH] = XT[b * 128:(b + 1) * 128]
        pfx[:, b, CH:] = WC[b * 128:(b + 1) * 128]
    pfx[0, 4, :CH] = 1.0
    pfx[0, 4, CH:] = b_f
    chc16 = np.ascontiguousarray(children[:CH, SIZE:])

    shared = {"sel8": sel.astype(fp8), "wdr": wdr, "wt1": wt1, "wt2": wt2,
              "wt3": wt3, "bias2": bias2.astype(fp8),
              "pfx": pfx.astype(bf16), "chc16": chc16}
    in_maps = []
    for c in range(NCORES):
        shard = children[c * T_LOC:(c + 1) * T_LOC, :SIZE].astype(fp8)
        # staged[t, p, j, i, d] = shard[t*2048 + j*256 + i*128 + p, d]
        staged = np.ascontiguousarray(
            shard.reshape(NT, NJ, 2, 128, SIZE).transpose(0, 3, 1, 2, 4))
        trk_loc = tr_h[c * B_LOC:(c + 1) * B_LOC]
        t_hi = trk_loc.astype(fp8).astype(np.float32)
        streams = {"th8": t_hi, "tl8": 16.0 * (trk_loc - t_hi)}
        # layout [(t,p), i, m] = stream[t*128 + m, i*128 + p]
        tmaps = {k: np.ascontiguousarray(
                     v.reshape(NT, 128, 2, 128).transpose(0, 3, 2, 1)
                 ).astype(fp8).reshape(NT * 128, 2, 128)
                 for k, v in streams.items()}
        in_maps.append({
            "ch8": staged.reshape(T_LOC // 2, 2 * SIZE),
            **tmaps,
            **shared,
        })

    return in_maps


# revision 96
# speedup vs baseline: 1.9575x; 1.0045x over previous
"""Trainium2 Bass kernel for the DependencyTreeLSTM node-reduction step.

Contract: kernel(**inputs) takes the FULL (unsharded) numpy inputs exactly as
produced by setup_inputs() and returns the FULL [B, 2*SIZE] float32 output.

Strategy (8 NeuronCores, data-parallel over the node axis, no collectives):
  - Each core owns B/8 = 2048 nodes (= 32768 children rows).
  - Only the h-half of `children` is needed in bulk (the c-half matters only
    for the first 16 rows, see below). It is staged **fp8 e4m3**: the
    children stream is ~2/3 of all HBM traffic and fp8 halves it; the
    per-node means average 16 children so the quantization noise largely
    cancels (measured ~9e-3 scale-relative max error, gate is 2e-2).
  - The children stream is split over the three DMA queues (SP-HWDGE,
    ACT-HWDGE, Pool-SWDGE), which transfer concurrently; each queue is
    internally FIFO so chunk order is hand-scheduled around need times.
  - Segment mean per 128-node tile, produced feature-major directly: 16
    DoubleRow fp8 matmuls with the children blocks stationary and a
    0/(1/16) selection strip moving -> PSUM holds mean^T (the /16 lives in
    the strip values), quantized to fp8 by one VectorE copy.
  - iou = [mean, tracking_h, 1] @ 16*[W_iou; W_iou_track; b_iou] runs on
    PE entirely as fp8 DoubleRow groups; the activation applies the 1/16.
    Tracking needs ~bf16 accuracy (plain fp8 fails the gate at 2.9e-2), so
    its GEMM is split into three fp8 rank-compensation groups with
    power-of-2 scale balancing: q(trk)@q(16W) + q(trk-q(trk))@q(16W) +
    q(trk)@q(16W-q(16W)) - measured equal to the bf16 path; both operand
    hi-streams are reused so only two tracking and two W-track streams
    ship over HBM.
  - sigmoid/tanh on ScalarE, elementwise on VectorE, node-major DMA out.
    The PE instruction stream is the wall-clock floor (hwdecode makes its
    per-instruction overhead ~2 ns); DMA is hand-scheduled across the
    three queues so every input lands just before its consumer.
  - The reference computes fc_b = cumsum(fc)[lens-1]; with lens == 16
    everywhere this is one shared prefix over the first 16 rows of fc.
    Each core recomputes that tiny [1, 256] vector on device and
    broadcasts it with a K=1 ones outer-product matmul.

If the inputs do not match the structural assumptions (uniform 16-child
segments), we fall back to a plain numpy implementation of the reference
(never taken for the benchmark inputs).
"""

import sys

if "/opt/trn_rl_repo" not in sys.path:
    sys.path.insert(0, "/opt/trn_rl_repo")

import numpy as np

B = 16384
CH = 16
T = B * CH
SIZE = 256
TR = 256
NCORES = 8
B_LOC = B // NCORES          # 2048 nodes per core
T_LOC = B_LOC * CH           # 32768 children rows per core
NT = B_LOC // 128            # 16 node-tiles of 128 nodes per core
NJ = 8                       # DoubleRow matmuls per node-tile (256 rows each)
# children chunks: (queue, tiles). The sync HWDGE queue takes the earliest
# tiles (fast handoff); the SWDGE queue takes the bulk that is needed later.
CH_CHUNKS = [("sync", [0]), ("gpsimd", [1]), ("sync", [2, 3]),
             ("sync", [4, 5]), ("gpsimd", [6, 7]), ("gpsimd", [8, 9]),
             ("sync", [10, 11]), ("gpsimd", [12, 13]), ("sync", [14]),
             ("gpsimd", [15])]
# tile index -> chunk ids whose DMA fires at the top of that tile
CH_ISSUE = {0: [0, 1], 1: [2], 2: [3, 4], 5: [5], 7: [6], 8: [7], 9: [9],
            11: [8]}
TRK_SPLIT = 3                # tracking tiles on sync before the rest on pool
OUT_PLAN = [4, 4, 4, 3, 1]   # output DMA groups; small tail groups
Y_ISSUE = {6: 0, 10: 1, 12: 2, 14: 3, 15: 4}  # tile at which y-group fires

_cache = {}


def _sigmoid(x):
    return 1.0 / (1.0 + np.exp(-x))


def _reference_np(children, tracking, W_iou, b_iou, W_f, b_f, W_iou_track,
                  W_f_track, segment_ids, lens):
    size = W_f.shape[0]
    nb = tracking.shape[0]
    tr_h = tracking[:, : tracking.shape[1] // 2]
    sums = np.zeros((nb, children.shape[1]), np.float32)
    np.add.at(sums, segment_ids, children)
    mean_h = (sums / lens[:, None].astype(np.float32))[:, :size]
    iou = mean_h @ W_iou + b_iou + tr_h @ W_iou_track
    i, o, u = np.split(iou, 3, axis=1)
    i, o, u = _sigmoid(i), _sigmoid(o), np.tanh(u)
    f = children[:, :size] @ W_f + b_f + (tr_h @ W_f_track)[segment_ids]
    fc = _sigmoid(f) * children[:, size:]
    cs = np.cumsum(fc, axis=0, dtype=np.float32)
    fc_b = cs[lens - 1]
    c = i * u + fc_b
    h = o * c
    return np.concatenate([h, c], axis=1).astype(np.float32)


def _build_nc():
    import concourse.tile as tile
    from concourse import bacc, mybir

    f32 = mybir.dt.float32
    f32r = mybir.dt.float32r
    bf16 = mybir.dt.bfloat16
    fp8 = mybir.dt.float8e4
    SIG = mybir.ActivationFunctionType.Sigmoid
    TANH = mybir.ActivationFunctionType.Tanh
    COPY = mybir.ActivationFunctionType.Copy
    DR = mybir.MatmulPerfMode.DoubleRow

    nc = bacc.Bacc("TRN2", target_bir_lowering=False, debug=False,
                   num_devices=NCORES)

    # children h-half, fp8, staged (t, p, j, i*d) so each partition's DMA
    # line is one contiguous 4 KB run per node-tile
    ch8 = nc.declare_dram_parameter("ch8", [T_LOC // 2, 2 * SIZE], fp8,
                                    isOutput=False)
    # tracking h transposed, two fp8 streams: hi = q(trk) and
    # lo = q(16*(trk - hi)); with the matching W streams the three DoubleRow
    # groups (hi reused for the W-residual group) reproduce
    # trk @ (16*W_track) to ~bf16 accuracy
    th8 = nc.declare_dram_parameter("th8", [NT * 128, 2, 128], fp8,
                                    isOutput=False)
    tl8 = nc.declare_dram_parameter("tl8", [NT * 128, 2, 128], fp8,
                                    isOutput=False)
    # selection strip: strip[p, i, x] = 1/16 iff x == 8i + p//16 + 112;
    # columns [240:368) hold the DR-bias lhs (partition 0: half0=1, half1=0)
    sel8 = nc.declare_dram_parameter("sel8", [128, 2, 368], fp8, isOutput=False)
    # W_iou as a DoubleRow K-group: wdr[p, i, n] = W_iou[i*128+p, n]
    wdr = nc.declare_dram_parameter("wdr", [128, 2, 3 * SIZE], fp8,
                                    isOutput=False)
    # W_iou_track fp8 K-groups: hi = q(16W), res = q(16W - hi); hi is
    # shared by the main and trk-residual groups
    wt1 = nc.declare_dram_parameter("wt1", [128, 2, 3 * SIZE], fp8,
                                    isOutput=False)
    wt3 = nc.declare_dram_parameter("wt3", [128, 2, 3 * SIZE], fp8,
                                    isOutput=False)
    # bias as DR rhs: row 0 = 16*b_iou, row 1 = 0 (lhs supplies the 1/16)
    bias2 = nc.declare_dram_parameter("bias2", [1, 2, 3 * SIZE], fp8,
                                      isOutput=False)
    # fc prefix inputs packed into one tensor: block b<4 holds the K-block
    # [X^T | Wcat] pair, block 4 row 0 holds the [ones | b_f] tail
    pfx = nc.declare_dram_parameter("pfx", [128, 5, CH + SIZE], bf16,
                                    isOutput=False)
    chc16 = nc.declare_dram_parameter("chc16", [CH, SIZE], f32, isOutput=False)
    y = nc.declare_dram_parameter("y", [B_LOC, 2 * SIZE], bf16, isOutput=True)

    chv = ch8[:].rearrange("(t p j) q -> p t j q", p=128, j=NJ)  # q = 2*256
    thv = th8[:].rearrange("(t p) i m -> p t i m", p=128)
    tlv = tl8[:].rearrange("(t p) i m -> p t i m", p=128)
    yv = y[:].rearrange("(t p) d -> p t d", p=128)
    assert sorted(sum((ts for _, ts in CH_CHUNKS), [])) == list(range(NT))
    assert sum(OUT_PLAN) == NT

    with tile.TileContext(nc) as tc:
        with (
            tc.tile_pool(name="consts", bufs=1) as consts,
            tc.tile_pool(name="chpool_a", bufs=3) as chpool_a,
            tc.tile_pool(name="chpool_b", bufs=3) as chpool_b,
            tc.tile_pool(name="ztpool", bufs=3) as ztpool,
            tc.tile_pool(name="actpool", bufs=3) as actpool,
            tc.tile_pool(name="outpool", bufs=2) as outpool,
            tc.tile_pool(name="psum_s", bufs=2, space="PSUM") as psum_s,
            tc.tile_pool(name="psum_i", bufs=3, space="PSUM") as psum_i,
        ):
            # ---- DMA schedule ----
            # sync  : ch{0}, ch{1}, trk[0:4], ch{4,5}, y0, ch{8,9}, y1,
            #         ch{14}, y2, ch{10,11}... (children issued in-loop)
            # gpsimd: sel, ch{2,3}, trk[4:16], ch{6,7}, ch{12,13}, ch{15}
            # scalar: pfx, chc16, wdr, wtk, bias
            th_sb = consts.tile([128, NT, 2, 128], fp8)
            tl_sb = consts.tile([128, NT, 2, 128], fp8)
            sel_sb = consts.tile([128, 2, 368], fp8)
            nc.gpsimd.dma_start(out=sel_sb, in_=sel8[:])
            wdr_sb = consts.tile([128, 2, 3 * SIZE], fp8)
            nc.scalar.dma_start(out=wdr_sb, in_=wdr[:])
            wt1_sb = consts.tile([128, 2, 3 * SIZE], fp8)
            nc.scalar.dma_start(out=wt1_sb, in_=wt1[:])
            wt3_sb = consts.tile([128, 2, 3 * SIZE], fp8)
            nc.scalar.dma_start(out=wt3_sb, in_=wt3[:])
            bias_sb = consts.tile([1, 2, 3 * SIZE], fp8)
            nc.scalar.dma_start(out=bias_sb, in_=bias2[:])
            pfx_sb = consts.tile([128, 5, CH + SIZE], bf16)
            chc_sb = consts.tile([CH, SIZE], f32)
            blhs_sb = sel_sb[0:1, :, 240:368]
            ones_f = consts.tile([CH, 128], f32)
            nc.vector.memset(ones_f, 1.0)
            ones_v = ones_f[:].bitcast(f32r)
            ones1 = ones_v[0:1, :]
            ones16 = ones_v[:, 0:1]

            bc_sb = consts.tile([128, SIZE], bf16)
            psum_f_box = []

            def emit_prefix():
                # fc prefix: fc_b = sum_{t<16} sigmoid(X @ Wcat)[t] * ch_c[t]
                # Emitted between tile 0's mean and its iou: the PE fills the
                # window where it would otherwise stall on the wtk/bias DMAs.
                psum_f = psum_i.tile([CH, SIZE], f32, tag="pi")
                for b in range(4):
                    nc.tensor.matmul(psum_f, lhsT=pfx_sb[:, b, 0:CH],
                                     rhs=pfx_sb[:, b, CH:],
                                     start=(b == 0), stop=False)
                nc.tensor.matmul(psum_f, lhsT=pfx_sb[0:1, 4, 0:CH],
                                 rhs=pfx_sb[0:1, 4, CH:],
                                 start=False, stop=True)
                psum_f_box.append(psum_f)

            def emit_prefix_reduce():
                psum_f = psum_f_box.pop()
                sig_sb = consts.tile([CH, SIZE], f32)
                nc.scalar.activation(out=sig_sb, in_=psum_f, func=SIG)
                fc_sb = consts.tile([CH, SIZE], f32r)
                nc.vector.tensor_mul(fc_sb, sig_sb, chc_sb)
                psum_pref = psum_i.tile([1, SIZE], f32, tag="pi")
                nc.tensor.matmul(psum_pref, lhsT=ones16,
                                 rhs=fc_sb[:], start=True, stop=True)
                pref_sb = consts.tile([1, SIZE], f32r)
                nc.vector.tensor_copy(pref_sb, psum_pref)
                psum_bc = psum_i.tile([128, SIZE], f32, tag="pi")
                nc.tensor.matmul(psum_bc, lhsT=ones1,
                                 rhs=pref_sb[:], start=True, stop=True)
                nc.vector.tensor_copy(bc_sb, psum_bc)

            # ---- main loop over node-tiles ----
            chunk_of = {}
            for ci, (q, ts) in enumerate(CH_CHUNKS):
                for hi, t in enumerate(ts):
                    chunk_of[t] = (ci, hi)
            nmax = max(len(ts) for _, ts in CH_CHUNKS)
            ogrp_of = []
            for ui, n in enumerate(OUT_PLAN):
                ogrp_of += [(ui, hi, n) for hi in range(n)]
            ogrp_starts = [sum(OUT_PLAN[:ui]) for ui in range(len(OUT_PLAN))]
            ch_sbs = {}
            out_grps = {}
            zt_sbs = {}

            def stage_sums(t):
                # fire children chunk DMAs scheduled at this tile index
                for ci in CH_ISSUE.get(t, ()):
                    q, ts = CH_CHUNKS[ci]
                    pool = chpool_a if q == "sync" else chpool_b
                    ch_sbn = pool.tile([128, nmax, NJ, 2 * SIZE],
                                       fp8, name=f"ch{ci}", tag=f"ch_{q}")
                    dma_eng = {"sync": nc.sync, "gpsimd": nc.gpsimd}[q]
                    if ci == 0:
                        # split so the very first half-tile lands sooner
                        dma_eng.dma_start(out=ch_sbn[:, 0, 0:NJ // 2],
                                          in_=chv[:, 0, 0:NJ // 2])
                        dma_eng.dma_start(out=ch_sbn[:, 0, NJ // 2:],
                                          in_=chv[:, 0, NJ // 2:])
                    else:
                        dma_eng.dma_start(out=ch_sbn[:, :len(ts)],
                                          in_=chv[:, ts[0]:ts[0] + len(ts)])
                    ch_sbs[ci] = ch_sbn
                # prefix consts + tracking ride the sync queue early
                if t == 0:
                    for sb, v in ((th_sb, thv), (tl_sb, tlv)):
                        nc.sync.dma_start(out=sb[:, 0:TRK_SPLIT],
                                          in_=v[:, 0:TRK_SPLIT])
                    nc.sync.dma_start(out=pfx_sb, in_=pfx[:])
                    nc.sync.dma_start(out=chc_sb, in_=chc16[:])
                elif t == 1:
                    for sb, v in ((th_sb, thv), (tl_sb, tlv)):
                        nc.gpsimd.dma_start(out=sb[:, TRK_SPLIT:NT],
                                            in_=v[:, TRK_SPLIT:NT])
                ci, hh = chunk_of[t]
                ch_sb = ch_sbs[ci][:, hh]          # [128, NJ, 512]

                # segment mean, produced feature-major directly: children
                # blocks stationary, selection strip (values 1/16) moving.
                # out[d, node] = sum over the block's 256 child rows; 16
                # DoubleRow fp8 matmuls accumulate all 2048 rows.
                psum_sum = psum_s.tile([128, 2, 128], f32, name=f"ps{t}",
                                       tag="ps")
                for j in range(NJ):
                    chj = ch_sb[:, j].rearrange("p (i d) -> p i d", i=2)
                    selj = sel_sb[:, :, 112 - 16 * j:240 - 16 * j]
                    for h in range(2):
                        nc.tensor.matmul(psum_sum[:, h],
                                         lhsT=chj[:, :, 128 * h:128 * h + 128],
                                         rhs=selj,
                                         start=(j == 0 and h == 0),
                                         stop=(j == NJ - 1 and h == 1),
                                         perf_mode=DR)
                zt_sb = ztpool.tile([128, 2, 128], fp8, name=f"zt{t}", tag="zt")
                nc.vector.tensor_copy(zt_sb, psum_sum)
                zt_sbs[t] = zt_sb
                if t == 1:
                    emit_prefix()

            def stage_rest(t):
                zt_sb = zt_sbs.pop(t)
                # iou[node, 0:768] = mean @ W_iou + trk @ W_iou_track + b_iou
                # PSUM groups are per 2KB zero-region (512 f32 cols): one
                # start and one stop per region; DR moving capped at 256 cols
                psum_iou = psum_i.tile([128, 3 * SIZE], f32, name=f"pi{t}",
                                       tag="pi")
                for ck in range(3):
                    ds = slice(256 * ck, 256 * ck + 256)
                    first = ck in (0, 2)
                    last = ck in (1, 2)
                    for gi, (lhs, rhs) in enumerate((
                            (th_sb[:, t], wt1_sb), (tl_sb[:, t], wt1_sb),
                            (th_sb[:, t], wt3_sb), (zt_sb, wdr_sb),
                            (blhs_sb, bias_sb))):
                        nc.tensor.matmul(psum_iou[:, ds], lhsT=lhs,
                                         rhs=rhs[:, :, ds],
                                         start=(first and gi == 0),
                                         stop=(last and gi == 4),
                                         perf_mode=DR)

                if t == 0:
                    emit_prefix_reduce()
                # PSUM holds 16x iou (weights staged pre-scaled by 16 so the
                # fp8 W_iou values stay clear of the subnormal range)
                act_sb = actpool.tile([128, 3 * SIZE], bf16, name=f"ac{t}",
                                      tag="ac")
                nc.scalar.activation(out=act_sb[:, 0:512],
                                     in_=psum_iou[:, 0:512], func=SIG,
                                     scale=1.0 / 16.0)
                nc.scalar.activation(out=act_sb[:, 512:768],
                                     in_=psum_iou[:, 512:768], func=TANH,
                                     scale=1.0 / 16.0)

                u, gh, gn = ogrp_of[t]
                if gh == 0:
                    out_grps[u] = outpool.tile([128, max(OUT_PLAN), 2 * SIZE],
                                               bf16, name=f"ot{u}", tag="ot")
                out_sb = out_grps[u][:, gh]
                # c = i*u + fc_b ; h = o*c
                nc.vector.tensor_mul(out_sb[:, 256:512], act_sb[:, 0:256],
                                     act_sb[:, 512:768])
                nc.vector.tensor_add(out_sb[:, 256:512], out_sb[:, 256:512],
                                     bc_sb)
                nc.vector.tensor_mul(out_sb[:, 0:256], act_sb[:, 256:512],
                                     out_sb[:, 256:512])
                # y groups ride the sync queue, issued late so they never
                # block a children chunk the compute still needs
                if t in Y_ISSUE:
                    u2 = Y_ISSUE[t]
                    g0 = ogrp_starts[u2]
                    gn2 = OUT_PLAN[u2]
                    # the last group rides the (idle) scalar queue so it
                    # does not wait behind the previous group's transfer
                    eng = nc.scalar if u2 == len(OUT_PLAN) - 2 else nc.sync
                    eng.dma_start(out=yv[:, g0:g0 + gn2],
                                  in_=out_grps[u2][:, :gn2])

            # software pipeline: sums run one tile ahead of iou/act/ew so PE
            # fills the initial weight-DMA wait with the next tile's sums
            for t in range(NT + 1):
                if t < NT:
                    stage_sums(t)
                if t >= 1:
                    stage_rest(t - 1)

    nc.finalize()
    return nc


def _get_nc():
    if "nc" not in _cache:
        _cache["nc"] = _build_nc()
    return _cache["nc"]


def kernel(**inputs):
    children = np.ascontiguousarray(np.asarray(inputs["children"], np.float32))
    tracking = np.ascontiguousarray(np.asarray(inputs["tracking"], np.float32))
    W_iou = np.asarray(inputs["W_iou"], np.float32)
    b_iou = np.asarray(inputs["b_iou"], np.float32)
    W_f = np.asarray(inputs["W_f"], np.float32)
    b_f = np.asarray(inputs["b_f"], np.float32)
    W_iou_track = np.asarray(inputs["W_iou_track"], np.float32)
    W_f_track = np.asarray(inputs["W_f_track"], np.float32)
    segment_ids = np.asarray(inputs["segment_ids"], np.int32)
    lens = np.asarray(inputs["lens"], np.int32)

    structured = (
        children.shape == (T, 2 * SIZE)
        and tracking.shape == (B, 2 * TR)
        and W_iou.shape == (SIZE, 3 * SIZE)
        and W_f.shape == (SIZE, SIZE)
        and W_iou_track.shape == (TR, 3 * SIZE)
        and W_f_track.shape == (TR, SIZE)
        and lens.shape == (B,)
        and segment_ids.shape == (T,)
        and bool((lens == CH).all())
        and bool((segment_ids == np.repeat(np.arange(B, dtype=np.int32), CH)).all())
    )
    if not structured:
        return _reference_np(children, tracking, W_iou, b_iou, W_f, b_f,
                             W_iou_track, W_f_track, segment_ids, lens)

    from concourse.bass_utils import run_bass_kernel_spmd

    nc = _get_nc()
    in_maps = _stage_in_maps(children, tracking, W_iou, b_iou, W_f, b_f,
                             W_iou_track, W_f_track, segment_ids)

    res = run_bass_kernel_spmd(nc, in_maps, core_ids=list(range(NCORES)))
    _cache["last_exec_time_ns"] = res.exec_time_ns
    out = np.concatenate([np.asarray(r["y"]).astype(np.float32)
                          for r in res.results], axis=0)
    return out


def _stage_in_maps(children, tracking, W_iou, b_iou, W_f, b_f,
                   W_iou_track, W_f_track, segment_ids):
    import ml_dtypes

    bf16 = ml_dtypes.bfloat16
    fp8 = ml_dtypes.float8_e4m3
    tr_h = tracking[:, :TR]

    # selection strip: strip[p, i, x] = 1/16 iff x == 8i + p//16 + 112, so
    # the slice strip[:, :, 112-16j : 240-16j] selects node 16j+8i+p//16
    p = np.arange(128)
    sel = np.zeros((128, 2, 368), np.float32)
    for i in range(2):
        sel[p, i, 8 * i + p // 16 + 112] = 1.0 / 16.0
    sel[0, 0, 240:368] = 1.0

    # weights pre-scaled by 16 (activation applies 1/16): keeps the fp8
    # W_iou values in the e4m3 normal range
    def kblocks(w):
        return np.ascontiguousarray(
            w.reshape(2, 128, 3 * SIZE).transpose(1, 0, 2)).astype(fp8)

    wdr = kblocks(16.0 * W_iou)
    WS = 16.0 * W_iou_track
    WS_hi = WS.astype(fp8).astype(np.float32)
    wt1 = kblocks(WS)
    wt3 = kblocks(WS - WS_hi)
    bias2 = np.zeros((1, 2, 3 * SIZE), np.float32)
    bias2[0, 0] = 16.0 * b_iou

    # prefix-f inputs: X = [ch_h[0:16], trk_h[seg[0:16]], 1],
    # W = [W_f; W_f_track; b_f], packed K-blocks [X^T | Wcat]
    X = np.concatenate([
        children[:CH, :SIZE],
        tr_h[segment_ids[:CH]],
        np.ones((CH, 1), np.float32),
    ], axis=1)                                       # [16, 513]
    XT = X.T                                         # [513, 16]
    WC = np.concatenate([W_f, W_f_track], axis=0)    # [512, 256]
    pfx = np.zeros((128, 5, CH + SIZE), np.float32)
    for b in range(4):
        pfx[:, b, :CH] = XT[b * 128:(b + 1) * 128]
        pfx[:, b, CH:] = WC[b * 128:(b + 1) * 128]
    pfx[0, 4, :CH] = 1.0
    pfx[0, 4, CH:] = b_f
    chc16 = np.ascontiguousarray(children[:CH, SIZE:])

    shared = {"sel8": sel.astype(fp8), "wdr": wdr, "wt1": wt1,
              "wt3": wt3, "bias2": bias2.astype(fp8),
              "pfx": pfx.astype(bf16), "chc16": chc16}
    in_maps = []
    for c in range(NCORES):
        shard = children[c * T_LOC:(c + 1) * T_LOC, :SIZE].astype(fp8)
        # staged[t, p, j, i, d] = shard[t*2048 + j*256 + i*128 + p, d]
        staged = np.ascontiguousarray(
            shard.reshape(NT, NJ, 2, 128, SIZE).transpose(0, 3, 1, 2, 4))
        trk_loc = tr_h[c * B_LOC:(c + 1) * B_LOC]
        t_hi = trk_loc.astype(fp8).astype(np.float32)
        streams = {"th8": t_hi, "tl8": trk_loc - t_hi}
        # layout [(t,p), i, m] = stream[t*128 + m, i*128 + p]
        tmaps = {k: np.ascontiguousarray(
                     v.reshape(NT, 128, 2, 128).transpose(0, 3, 2, 1)
                 ).astype(fp8).reshape(NT * 128, 2, 128)
                 for k, v in streams.items()}
        in_maps.append({
            "ch8": staged.reshape(T_LOC // 2, 2 * SIZE),
            **tmaps,
            **shared,
        })

    return in_maps
